# revision 1
# baseline (speedup 1.0000x reference)
"""ARGenerator TRN2 kernel builder.

Math (per batch row b):
  h1 = relu(x @ W1.T + b1); h2 = relu(h1 @ W2.T + b2)
  mlp = tanh(h2 @ W3.T + b3)
  ar[t] = noise[t] + sum_i c[i] * ar[t-1-i]  (zero-init, t >= 7; 0 for t < 7)
  out = mlp + ar

The AR recurrence is linear time-invariant -> ar = conv(noise_masked, h)
where h is the (geometrically decaying) impulse response and
noise_masked[t<7] = 0.  Truncating h at L taps makes the conv a banded
Toeplitz matmul, fully parallel over time.

Device layout strategy (per core shard of B rows):
  - MLP layer 1/2 in "transposed" activation layout (features on
    partitions, batch on free dim).  x tiles are transposed on the PE
    (is_transpose matmul), 2 b-subtiles at a time into a [128, CB] psum.
  - Layer 3 + conv emit NATURAL layout output ([128 batch, t free]):
      L3:   out_psum = h2T_slice.T @ W3T_chunk   (lhsT = h2T column slice)
      bias: out_psum += ones.T @ b3_chunk        (K=1 rank-1 matmul)
      conv: conv_psum = sum_j noiseT_tile(k).T @ Hband_j
    then ACT tanh(out_psum) -> sbuf, DVE add conv_psum -> out rows, DMA out.
All weights / bias / band matrices are pre-laid-out on the host.
"""

import math
import numpy as np

import concourse.bass as bass
import concourse.tile as tile
import concourse.mybir as mybir
from concourse import bacc

F32 = mybir.dt.float32
F32R = mybir.dt.float32r


def impulse_response(c, s_out, tail_tol=1e-6):
    """Return (h[:L], L) with L a multiple of 128, abs-tail below tail_tol."""
    AR = len(c)
    c = np.asarray(c, np.float64)
    h = np.zeros(s_out, np.float64)
    h[0] = 1.0
    for j in range(1, s_out):
        acc = 0.0
        for i in range(AR):
            if j - 1 - i >= 0:
                acc += c[i] * h[j - 1 - i]
        h[j] = acc
    L = 128
    while L < s_out and np.abs(h[L:]).sum() > tail_tol:
        L += 128
    return h, L


def band_tiles(h, L, FT):
    """j values and Hband matrix [128, len(js)*FT] with
    Hband[:, jj*FT:][a, b] = h[b - a - 128*j]."""
    n_lower = (L + 126) // 128
    js = []
    for j in range(-n_lower, FT // 128 + 1):
        lo = -127 - 128 * j
        hi = FT - 1 - 128 * j
        if hi >= 0 and lo <= L - 1:
            js.append(j)
    a = np.arange(128)[:, None]
    b = np.arange(FT)[None, :]
    blocks = []
    for j in js:
        idx = b - a - 128 * j
        m = (idx >= 0) & (idx < L)
        blk = np.where(m, np.take(np.pad(h[:L], (0, 1)), np.clip(idx, 0, L)), 0.0)
        blocks.append(blk)
    return js, np.concatenate(blocks, axis=1).astype(np.float32)


def host_prepare(W1, b1, W2, b2, W3, b3, ar_coef, S_IN, S_OUT, H, FT):
    """Build all the small device tensors in exactly the SBUF layout used."""
    n_s = S_IN // 128
    # W1Tl[p, k*H + h] = W1[h, k*128 + p]  (lhsT tiles for layer 1)
    W1Tl = np.ascontiguousarray(
        W1.reshape(H, n_s, 128).transpose(2, 1, 0).reshape(128, n_s * H)
    )
    W2T = np.ascontiguousarray(W2.T)  # [H_in, H_out]
    W3T = np.ascontiguousarray(W3.T)  # [H, S_OUT] (rhs for layer 3)
    h, L = impulse_response(ar_coef, S_OUT)
    js, Hband = band_tiles(h, L, FT)
    return {
        "W1Tl": W1Tl.astype(np.float32),
        "W2T": W2T.astype(np.float32),
        "W3T": W3T.astype(np.float32),
        "b1c": np.ascontiguousarray(b1.reshape(H, 1), dtype=np.float32),
        "b2c": np.ascontiguousarray(b2.reshape(H, 1), dtype=np.float32),
        "b3c": np.ascontiguousarray(b3.reshape(1, S_OUT), dtype=np.float32),
        "onesr": np.ones((1, 128), np.float32),
        "ident": np.eye(128, dtype=np.float32),
        "Hband": Hband,
    }, js


def build_kernel(B_shard, S_IN, S_OUT, H, js, CB=512, FT=256, use_f32r=True,
                 natw=2048):
    """v3: CB=512 chunks, F=512 streams, 4-transpose psum groups."""
    P = 128
    assert H == 128 and B_shard % CB == 0 and CB == 4 * P and FT == 256
    n_chunk = B_shard // CB
    nbs = CB // P                    # 4 b-subtiles per chunk
    n_s = S_IN // P                  # 32
    n_c5 = S_OUT // 512              # 8 output 512-chunks
    n_k = S_OUT // P                 # 32
    nat_h = S_IN // natw             # strips per row
    OW = min(2048, S_OUT)            # orow width
    ng = OW // 512                   # c5 groups per orow

    MD = F32R if use_f32r else F32

    def r(ap):
        return ap.bitcast(F32R) if use_f32r else ap

    nc = bacc.Bacc(trn_type="TRN2", target_bir_lowering=False, debug=False)

    x_d = nc.dram_tensor("x", [B_shard, S_IN], MD, kind="ExternalInput").ap()
    nz_d = nc.dram_tensor("noise_m", [B_shard, S_OUT], MD, kind="ExternalInput").ap()
    W1_d = nc.dram_tensor("W1Tl", [P, n_s * H], MD, kind="ExternalInput").ap()
    W2_d = nc.dram_tensor("W2T", [H, H], MD, kind="ExternalInput").ap()
    W3_d = nc.dram_tensor("W3T", [H, S_OUT], MD, kind="ExternalInput").ap()
    b1_d = nc.dram_tensor("b1c", [H, 1], F32, kind="ExternalInput").ap()
    b2_d = nc.dram_tensor("b2c", [H, 1], F32, kind="ExternalInput").ap()
    b3_d = nc.dram_tensor("b3c", [1, S_OUT], MD, kind="ExternalInput").ap()
    on_d = nc.dram_tensor("onesr", [1, P], MD, kind="ExternalInput").ap()
    id_d = nc.dram_tensor("ident", [P, P], MD, kind="ExternalInput").ap()
    Hb_d = nc.dram_tensor("Hband", [P, len(js) * FT], MD, kind="ExternalInput").ap()
    out_d = nc.dram_tensor("out", [B_shard, S_OUT], F32, kind="ExternalOutput").ap()

    with tile.TileContext(nc) as tc:
        with tc.tile_pool(name="const", bufs=1) as cpool:
            ids = cpool.tile([P, P], MD, tag="ident")
            nc.sync.dma_start(ids[:], id_d[:])
            W1s = cpool.tile([P, n_s * H], MD, tag="w1")
            nc.sync.dma_start(W1s[:], W1_d[:])
            W2s = cpool.tile([H, H], MD, tag="w2")
            nc.scalar.dma_start(W2s[:], W2_d[:])
            W3s = cpool.tile([H, S_OUT], MD, tag="w3")
            nc.scalar.dma_start(W3s[:], W3_d[:])
            b1s = cpool.tile([H, 1], F32, tag="b1")
            nc.scalar.dma_start(b1s[:], b1_d[:])
            b2s = cpool.tile([H, 1], F32, tag="b2")
            nc.scalar.dma_start(b2s[:], b2_d[:])
            b3s = cpool.tile([1, S_OUT], MD, tag="b3")
            nc.scalar.dma_start(b3s[:], b3_d[:])
            ons = cpool.tile([1, P], MD, tag="ones")
            nc.scalar.dma_start(ons[:], on_d[:])
            Hbs = cpool.tile([P, len(js) * FT], MD, tag="hband")
            nc.scalar.dma_start(Hbs[:], Hb_d[:])

            with (
                tc.tile_pool(name="natx", bufs=8) as natxp,
                tc.tile_pool(name="natn", bufs=8) as natnp,
                tc.tile_pool(name="xT", bufs=6) as xTp,
                tc.tile_pool(name="nT", bufs=8) as nTp,
                tc.tile_pool(name="act", bufs=4) as actp,
                tc.tile_pool(name="orow", bufs=4) as orowp,
                tc.tile_pool(name="ps_t", bufs=3, space="PSUM") as ps_t,
                tc.tile_pool(name="ps_big", bufs=5, space="PSUM") as ps_big,
            ):
                kst = natw // P  # k tiles per strip
                for cc in range(n_chunk):
                    b0 = cc * CB
                    # strips indexed [i][h]
                    xrows = [[None] * nat_h for _ in range(nbs)]

                    def load_strip(rows, dram, i, hh, pfx, pool):
                        t = pool.tile([P, natw], MD, tag="nat",
                                      name=f"{pfx}{cc}_{i}_{hh}")
                        nc.sync.dma_start(
                            t[:],
                            dram[b0 + i * P:b0 + (i + 1) * P,
                                 hh * natw:(hh + 1) * natw],
                        )
                        rows[i][hh] = t

                    for i in range(nbs):
                        load_strip(xrows, x_d, i, 0, "xr", natxp)
                    psh1 = ps_big.tile([H, CB], F32, tag="ps", name=f"psh1_{cc}")
                    for k in range(n_s):
                        hh, off = divmod(k, kst)
                        if off == 0 and hh + 1 < nat_h:
                            for i in range(nbs):
                                load_strip(xrows, x_d, i, hh + 1, "xr", natxp)
                        pst = ps_t.tile([P, CB], F32, tag="pst")
                        for i in range(nbs):
                            nc.tensor.matmul(
                                r(pst[:, i * P:(i + 1) * P]),
                                xrows[i][hh][:, off * P:(off + 1) * P],
                                ids[:],
                                start=(i == 0),
                                stop=(i == nbs - 1),
                                is_transpose=True,
                            )
                        xT = xTp.tile([P, CB], MD, tag="xT")
                        if k % 2 == 0:
                            nc.vector.tensor_copy(xT[:], r(pst[:]))
                        else:
                            nc.scalar.copy(xT[:], pst[:])
                        nc.tensor.matmul(
                            psh1[:],
                            W1s[:, k * H:(k + 1) * H],
                            xT[:],
                            start=(k == 0),
                            stop=(k == n_s - 1),
                        )
                    h1T = actp.tile([H, CB], MD, tag="h1T", bufs=2)
                    nc.scalar.activation(
                        h1T[:], psh1[:], mybir.ActivationFunctionType.Relu,
                        bias=b1s[:],
                    )
                    psh2 = ps_big.tile([H, CB], F32, tag="ps", name=f"psh2_{cc}")
                    nc.tensor.matmul(psh2[:], W2s[:], h1T[:])
                    h2T = actp.tile([H, CB], MD, tag="h2T", bufs=2)
                    nc.scalar.activation(
                        h2T[:], psh2[:], mybir.ActivationFunctionType.Relu,
                        bias=b2s[:],
                    )

                    nrows = [[None] * nat_h for _ in range(nbs)]
                    for i in range(nbs):
                        load_strip(nrows, nz_d, i, 0, "nr", natnp)
                    orows = {}
                    nT = {}

                    def emit_noise_tiles(k_lo, k_hi):
                        for k in range(k_lo, min(k_hi, n_k)):
                            hh, off = divmod(k, kst)
                            if off == 0 and hh + 1 < nat_h:
                                for i2 in range(nbs):
                                    load_strip(nrows, nz_d, i2, hh + 1, "nr", natnp)
                            pstn = ps_t.tile([P, CB], F32, tag="pst",
                                             name=f"pstn{cc}_{k}")
                            for i2 in range(nbs):
                                nc.tensor.matmul(
                                    r(pstn[:, i2 * P:(i2 + 1) * P]),
                                    nrows[i2][hh][:, off * P:(off + 1) * P],
                                    ids[:],
                                    start=(i2 == 0),
                                    stop=(i2 == nbs - 1),
                                    is_transpose=True,
                                )
                            nT[k] = nTp.tile([P, CB], MD, tag="nT",
                                             name=f"nT{cc}_{k}")
                            nc.vector.tensor_copy(nT[k][:], r(pstn[:]))

                    emit_noise_tiles(0, 8)
                    for c5 in range(n_c5):
                        if c5 > 0:
                            emit_noise_tiles(4 * c5 + 4, 4 * c5 + 8)
                        if c5 % ng == 0:
                            for i in range(nbs):
                                orows[i, c5 // ng] = orowp.tile(
                                    [P, OW], F32, tag="orow",
                                    name=f"orow{cc}_{i}_{c5 // ng}")
                        for i in range(nbs):
                            psm = ps_big.tile([P, 512], F32, tag="ps", name=f"psm{cc}_{c5}_{i}")
                            nc.tensor.matmul(
                                psm[:],
                                h2T[:, i * P:(i + 1) * P],
                                W3s[:, c5 * 512:(c5 + 1) * 512],
                                start=True,
                                stop=False,
                            )
                            nc.tensor.matmul(
                                psm[:],
                                ons[:],
                                b3s[:, c5 * 512:(c5 + 1) * 512],
                                start=False,
                                stop=True,
                            )
                            psc = ps_big.tile([P, 512], F32, tag="ps", name=f"psc{cc}_{c5}_{i}")
                            mms = []
                            for ch in range(2):
                                c = 2 * c5 + ch
                                for j in js:
                                    k = c * 2 + j
                                    if 0 <= k < n_k:
                                        mms.append((ch, js.index(j), k))
                            for m, (ch, jj, k) in enumerate(mms):
                                nc.tensor.matmul(
                                    psc[:, ch * FT:(ch + 1) * FT],
                                    nT[k][:, i * P:(i + 1) * P],
                                    Hbs[:, jj * FT:(jj + 1) * FT],
                                    start=(m == 0),
                                    stop=(m == len(mms) - 1),
                                )
                            th = actp.tile([P, 512], F32, tag="th")
                            nc.scalar.activation(
                                th[:], psm[:], mybir.ActivationFunctionType.Tanh
                            )
                            nc.vector.tensor_add(
                                orows[i, c5 // ng][:, (c5 % ng) * 512:
                                                   (c5 % ng + 1) * 512],
                                th[:], psc[:],
                            )
                        if c5 % ng == ng - 1:
                            for i in range(nbs):
                                nc.sync.dma_start(
                                    out_d[b0 + i * P:b0 + (i + 1) * P,
                                          (c5 // ng) * OW:
                                          (c5 // ng + 1) * OW],
                                    orows[i, c5 // ng][:],
                                )

    nc.compile()
    return nc


# ---------------------------------------------------------------------------
# Self-contained kernel() entry point (the graded contract).
# ---------------------------------------------------------------------------

N_CORES = 8
_B, _S_IN, _S_OUT, _H, _AR = 8192, 4096, 4096, 128, 7
_FT = 256
_USE_F32R = True

_CACHE = {}


def _prep_and_build(inputs):
    dev, js = host_prepare(
        np.asarray(inputs["W1"], np.float32), np.asarray(inputs["b1"], np.float32),
        np.asarray(inputs["W2"], np.float32), np.asarray(inputs["b2"], np.float32),
        np.asarray(inputs["W3"], np.float32), np.asarray(inputs["b3"], np.float32),
        np.asarray(inputs["ar_coef"], np.float32),
        _S_IN, _S_OUT, _H, _FT,
    )
    B_total = inputs["x"].shape[0]
    B_shard = B_total // N_CORES
    key = (B_shard, tuple(js), _USE_F32R)
    if key not in _CACHE:
        _CACHE[key] = build_kernel(
            B_shard, _S_IN, _S_OUT, _H, js, CB=512, FT=_FT,
            use_f32r=_USE_F32R, natw=1024,
        )
    return _CACHE[key], dev, B_shard


def _in_maps(inputs, dev, B_shard):
    x = np.ascontiguousarray(np.asarray(inputs["x"], np.float32))
    noise_m = np.asarray(inputs["noise"], np.float32).copy()
    noise_m[:, :_AR] = 0.0
    maps = []
    for c in range(N_CORES):
        sl = slice(c * B_shard, (c + 1) * B_shard)
        m = {"x": np.ascontiguousarray(x[sl]),
             "noise_m": np.ascontiguousarray(noise_m[sl])}
        m.update(dev)
        maps.append(m)
    return maps


def kernel(**inputs):
    nc, dev, B_shard = _prep_and_build(inputs)
    maps = _in_maps(inputs, dev, B_shard)
    import concourse.bass_utils as bass_utils

    res = bass_utils.run_bass_kernel_spmd(
        nc, maps, core_ids=list(range(N_CORES)), trace=False
    )
    return np.concatenate(
        [res.results[c]["out"] for c in range(N_CORES)], axis=0
    ).astype(np.float32)


def run_traced(inputs):
    """Profiled run (NTFF -> neuron-profile) for the local test harness."""
    import contextlib
    import ctypes
    import sys as _sys
    import types as _types

    so = "/opt/axon/libaxon_pjrt.so"
    if "antenv.axon_hooks" not in _sys.modules:
        try:
            lib2 = ctypes.CDLL(so)
            lib2.axon_start_nrt_profile.argtypes = [
                ctypes.POINTER(ctypes.c_int64), ctypes.c_size_t]
            lib2.axon_start_nrt_profile.restype = ctypes.c_int64
            lib2.axon_stop_nrt_profile.argtypes = [ctypes.c_char_p]
            lib2.axon_stop_nrt_profile.restype = ctypes.c_int64

            @contextlib.contextmanager
            def _hook(output_dir, device_ids):
                import jax
                jax.devices()
                if device_ids:
                    ids_arr = (ctypes.c_int64 * len(device_ids))(*device_ids)
                    rc = lib2.axon_start_nrt_profile(ids_arr, len(device_ids))
                else:
                    rc = lib2.axon_start_nrt_profile(None, 0)
                if rc != 0:
                    raise RuntimeError(f"axon_start_nrt_profile rc={rc}")
                try:
                    yield
                finally:
                    lib2.axon_stop_nrt_profile(str(output_dir).encode())

            mod = _types.ModuleType("antenv.axon_hooks")
            mod.get_axon_ntff_profile_hook = lambda: _hook
            mod.set_axon_ntff_profile_hook = lambda h: None
            _sys.modules["antenv.axon_hooks"] = mod
        except OSError:
            pass
    import concourse.bass_utils as bass_utils
    bass_utils.upload_artifacts = lambda tmpdir: tmpdir

    nc, dev, B_shard = _prep_and_build(inputs)
    maps = _in_maps(inputs, dev, B_shard)
    return bass_utils.run_bass_kernel_spmd(
        nc, maps, core_ids=list(range(N_CORES)), trace=True, trace_cores=[0]
    )



# revision 3
# speedup vs baseline: 1.7529x; 1.7529x over previous
"""ARGenerator TRN2 kernel.

Math (per batch row b):
  h1 = relu(x @ W1.T + b1); h2 = relu(h1 @ W2.T + b2)
  mlp = tanh(h2 @ W3.T + b3)
  ar[t] = noise[t] + sum_i c[i] * ar[t-1-i]  (zero-init, t >= 7; 0 for t < 7)
  out = mlp + ar

The AR recurrence is linear time-invariant -> ar = conv(noise_masked, h)
with h the (geometrically decaying) impulse response, truncated at
L = nb*128 taps.  The conv becomes nb banded 128x128 Toeplitz matmuls
per output time-tile, fully parallel over time.

Layout strategy (pure data parallel over 8 cores, B_shard = 1024):
  Everything runs in bf16 (tolerance is 2e-2; bf16 keeps us ~5e-3) and
  in TRANSPOSED activation layout [feature/time on partitions, batch on
  free dim].  The host pre-transposes x and noise shards to [S, B_shard]
  bf16, so the device does plain contiguous DMA loads and zero PE
  transposes.  The output is produced transposed [S_OUT, B_shard] bf16
  (which lets the b3 bias + tanh fuse into one ACT instruction with a
  per-partition bias) and the host transposes it back.

Per-core phases:
  1. L1: psum[h, b] += W1l_k.T @ xT_k over 32 k-tiles; ACT relu+b1.
  2. L2: one matmul + ACT relu+b2 -> h2T [128, 1024] bf16.
  3. For each t-tile m (32):
       psm = W3l[:, m].T @ h2T          (128x1024, via 2 512-halves)
       psc = sum_j Hb_j.T @ nT[m+j]     (banded conv, nb matmuls/half)
       th  = tanh(psm + b3[m])          (ACT, per-partition bias)
       outT_m = th + psc  (DVE) -> bf16 -> DMA store.
DMA per core: 1MB W1 + 1MB W3 + 8MB xT + 8MB nT + 8MB out ~ 26MB.
"""

import numpy as np
import ml_dtypes

import concourse.bass as bass
import concourse.tile as tile
import concourse.mybir as mybir
from concourse import bacc

F32 = mybir.dt.float32
BF16 = mybir.dt.bfloat16
BF16_NP = ml_dtypes.bfloat16


def impulse_response(c, s_out, tail_tol=1e-4):
    """Return (h, nb) with nb*128 taps covering the response to tail_tol."""
    AR = len(c)
    c = np.asarray(c, np.float64)
    h = np.zeros(s_out, np.float64)
    h[0] = 1.0
    for j in range(1, s_out):
        acc = 0.0
        for i in range(AR):
            if j - 1 - i >= 0:
                acc += c[i] * h[j - 1 - i]
        h[j] = acc
    L = 128
    while L < s_out and np.abs(h[L:]).sum() > tail_tol:
        L += 128
    # nb = number of 128-wide band blocks per output tile: the in-tile block
    # (j=0) plus one per preceding input tile the L-tap history reaches into.
    return h, L // 128 + 1


def band_blocks(h, nb):
    """Hb [128, nb*128]: block jj (for input-tile offset j = jj - (nb-1))
    has Hb[k_rel, t_rel] = h[t_rel - k_rel - 128*j] (0 <= lag < (nb-1)*128)."""
    L = (nb - 1) * 128
    a = np.arange(128)[:, None]   # k_rel
    b = np.arange(128)[None, :]   # t_rel
    blocks = []
    for jj in range(nb):
        j = jj - (nb - 1)
        lag = b - a - 128 * j
        m = (lag >= 0) & (lag < L)
        blk = np.where(m, np.take(np.pad(h[:L], (0, 1)), np.clip(lag, 0, L)), 0.0)
        blocks.append(blk)
    return np.concatenate(blocks, axis=1)


def host_prepare(W1, b1, W2, b2, W3, b3, ar_coef, S_IN, S_OUT, H):
    """Small device tensors in exactly the SBUF layout used, bf16."""
    n_s = S_IN // 128
    # W1l[p, k*H + h] = W1[h, k*128 + p]  (lhsT tiles for layer 1)
    W1l = np.ascontiguousarray(
        W1.reshape(H, n_s, 128).transpose(2, 1, 0).reshape(128, n_s * H)
    )
    h, nb = impulse_response(ar_coef, S_OUT)
    return {
        "W1l": W1l.astype(BF16_NP),
        "W2l": np.ascontiguousarray(W2.T).astype(BF16_NP),   # [H_in, H_out]
        "W3l": np.ascontiguousarray(W3.T).astype(BF16_NP),   # [H, S_OUT]
        "b1c": np.ascontiguousarray(b1.reshape(H, 1), dtype=np.float32),
        "b2c": np.ascontiguousarray(b2.reshape(H, 1), dtype=np.float32),
        "b3m": np.ascontiguousarray(b3.reshape(S_OUT // 128, 128).T,
                                    dtype=np.float32),       # [128, n_t]
        "Hb": band_blocks(h, nb).astype(BF16_NP),
    }, nb


def build_kernel(B_shard, S_IN, S_OUT, H, nb):
    P = 128
    HF = 512                      # matmul moving width (one PSUM bank)
    assert H == P and B_shard % HF == 0
    n_s = S_IN // P               # 32 input k-tiles
    n_t = S_OUT // P              # 32 output t-tiles
    n_h = B_shard // HF           # batch halves

    nc = bacc.Bacc(trn_type="TRN2", target_bir_lowering=False, debug=False)

    xT_d = nc.dram_tensor("xT", [S_IN, B_shard], BF16, kind="ExternalInput").ap()
    nT_d = nc.dram_tensor("nT", [S_OUT, B_shard], BF16, kind="ExternalInput").ap()
    W1_d = nc.dram_tensor("W1l", [P, n_s * H], BF16, kind="ExternalInput").ap()
    W2_d = nc.dram_tensor("W2l", [H, H], BF16, kind="ExternalInput").ap()
    W3_d = nc.dram_tensor("W3l", [H, S_OUT], BF16, kind="ExternalInput").ap()
    b1_d = nc.dram_tensor("b1c", [H, 1], F32, kind="ExternalInput").ap()
    b2_d = nc.dram_tensor("b2c", [H, 1], F32, kind="ExternalInput").ap()
    b3_d = nc.dram_tensor("b3m", [P, n_t], F32, kind="ExternalInput").ap()
    Hb_d = nc.dram_tensor("Hb", [P, nb * P], BF16, kind="ExternalInput").ap()
    out_d = nc.dram_tensor("outT", [S_OUT, B_shard], BF16,
                           kind="ExternalOutput").ap()

    with tile.TileContext(nc) as tc:
        with tc.tile_pool(name="const", bufs=1) as cpool:
            # sync queue: W1 first (needed first), then the x tiles below.
            W1s = cpool.tile([P, n_s * H], BF16, tag="w1")
            nc.sync.dma_start(W1s[:], W1_d[:])
            # scalar queue: everything needed later.
            W2s = cpool.tile([H, H], BF16, tag="w2")
            nc.scalar.dma_start(W2s[:], W2_d[:])
            b1s = cpool.tile([H, 1], F32, tag="b1")
            nc.scalar.dma_start(b1s[:], b1_d[:])
            b2s = cpool.tile([H, 1], F32, tag="b2")
            nc.scalar.dma_start(b2s[:], b2_d[:])
            W3s = cpool.tile([H, S_OUT], BF16, tag="w3")
            nc.scalar.dma_start(W3s[:], W3_d[:])
            b3s = cpool.tile([P, n_t], F32, tag="b3")
            nc.scalar.dma_start(b3s[:], b3_d[:])
            Hbs = cpool.tile([P, nb * P], BF16, tag="hb")
            nc.scalar.dma_start(Hbs[:], Hb_d[:])

            with (
                tc.tile_pool(name="xT", bufs=n_s) as xTp,
                tc.tile_pool(name="nT", bufs=4) as nTp,
                tc.tile_pool(name="act", bufs=2) as actp,
                tc.tile_pool(name="th", bufs=2) as thp,
                tc.tile_pool(name="outT", bufs=3) as outp,
                tc.tile_pool(name="ps", bufs=4, space="PSUM") as psp,
            ):
                # ---- load all 32 xT tiles (resident; both halves use them)
                xts = []
                for k in range(n_s):
                    t = xTp.tile([P, B_shard], BF16, tag="xt", name=f"xt{k}")
                    nc.sync.dma_start(t[:], xT_d[k * P:(k + 1) * P, :])
                    xts.append(t)
                # ---- noise tiles stream behind on the same queue
                nts = [None] * n_t

                def load_nt(m):
                    nts[m] = nTp.tile([P, B_shard], BF16, tag="nt",
                                      name=f"nt{m}")
                    nc.sync.dma_start(nts[m][:], nT_d[m * P:(m + 1) * P, :])

                load_nt(0)
                load_nt(1)

                # ---- layer 1: psh1[h, b] = sum_k W1l_k.T @ xT_k
                psh1 = psp.tile([H, B_shard], F32, tag="ps", name="psh1")
                for hh in range(n_h):
                    sl = slice(hh * HF, (hh + 1) * HF)
                    for k in range(n_s):
                        nc.tensor.matmul(
                            psh1[:, sl],
                            W1s[:, k * H:(k + 1) * H],
                            xts[k][:, sl],
                            start=(k == 0),
                            stop=(k == n_s - 1),
                        )
                h1T = actp.tile([H, B_shard], BF16, tag="h1T")
                nc.scalar.activation(
                    h1T[:], psh1[:], mybir.ActivationFunctionType.Relu,
                    bias=b1s[:],
                )
                # ---- layer 2
                psh2 = psp.tile([H, B_shard], F32, tag="ps", name="psh2")
                for hh in range(n_h):
                    sl = slice(hh * HF, (hh + 1) * HF)
                    nc.tensor.matmul(psh2[:, sl], W2s[:], h1T[:, sl])
                h2T = actp.tile([H, B_shard], BF16, tag="h2T")
                nc.scalar.activation(
                    h2T[:], psh2[:], mybir.ActivationFunctionType.Relu,
                    bias=b2s[:],
                )

                # ---- per t-tile: L3 matmul + banded conv + tanh/bias + add
                for m in range(n_t):
                    if m + 2 < n_t:
                        load_nt(m + 2)
                    psc = psp.tile([P, B_shard], F32, tag="ps", name=f"psc{m}")
                    jlist = [j for j in range(-(nb - 1), 1) if m + j >= 0]
                    for hh in range(n_h):
                        sl = slice(hh * HF, (hh + 1) * HF)
                        for i, j in enumerate(jlist):
                            jj = j + nb - 1
                            nc.tensor.matmul(
                                psc[:, sl],
                                Hbs[:, jj * P:(jj + 1) * P],
                                nts[m + j][:, sl],
                                start=(i == 0),
                                stop=(i == len(jlist) - 1),
                            )
                    psm = psp.tile([P, B_shard], F32, tag="ps", name=f"psm{m}")
                    for hh in range(n_h):
                        sl = slice(hh * HF, (hh + 1) * HF)
                        nc.tensor.matmul(
                            psm[:, sl],
                            W3s[:, m * P:(m + 1) * P],
                            h2T[:, sl],
                        )
                    th = thp.tile([P, B_shard], F32, tag="th")
                    nc.scalar.activation(
                        th[:], psm[:], mybir.ActivationFunctionType.Tanh,
                        bias=b3s[:, m:m + 1],
                    )
                    ot = outp.tile([P, B_shard], BF16, tag="ot", name=f"ot{m}")
                    nc.vector.tensor_add(ot[:], th[:], psc[:])
                    nc.scalar.dma_start(out_d[m * P:(m + 1) * P, :], ot[:])

    nc.compile()
    return nc


# ---------------------------------------------------------------------------
# Self-contained kernel() entry point (the graded contract).
# ---------------------------------------------------------------------------

N_CORES = 8
_B, _S_IN, _S_OUT, _H, _AR = 8192, 4096, 4096, 128, 7

_CACHE = {}


def _prep_and_build(inputs):
    dev, nb = host_prepare(
        np.asarray(inputs["W1"], np.float32), np.asarray(inputs["b1"], np.float32),
        np.asarray(inputs["W2"], np.float32), np.asarray(inputs["b2"], np.float32),
        np.asarray(inputs["W3"], np.float32), np.asarray(inputs["b3"], np.float32),
        np.asarray(inputs["ar_coef"], np.float32),
        _S_IN, _S_OUT, _H,
    )
    B_total = inputs["x"].shape[0]
    B_shard = B_total // N_CORES
    key = (B_shard, nb)
    if key not in _CACHE:
        _CACHE[key] = build_kernel(B_shard, _S_IN, _S_OUT, _H, nb)
    return _CACHE[key], dev, B_shard


def _in_maps(inputs, dev, B_shard):
    x = np.asarray(inputs["x"], np.float32)
    noise_m = np.asarray(inputs["noise"], np.float32).copy()
    noise_m[:, :_AR] = 0.0
    maps = []
    for c in range(N_CORES):
        sl = slice(c * B_shard, (c + 1) * B_shard)
        m = {"xT": np.ascontiguousarray(x[sl].astype(BF16_NP).T),
             "nT": np.ascontiguousarray(noise_m[sl].astype(BF16_NP).T)}
        m.update(dev)
        maps.append(m)
    return maps


def kernel(**inputs):
    nc, dev, B_shard = _prep_and_build(inputs)
    maps = _in_maps(inputs, dev, B_shard)
    import concourse.bass_utils as bass_utils

    res = bass_utils.run_bass_kernel_spmd(
        nc, maps, core_ids=list(range(N_CORES)), trace=False
    )
    return np.concatenate(
        [np.asarray(res.results[c]["outT"]).T for c in range(N_CORES)], axis=0
    ).astype(np.float32)


def run_traced(inputs):
    """Profiled run (NTFF -> neuron-profile) for the local test harness."""
    import contextlib
    import ctypes
    import sys as _sys
    import types as _types

    so = "/opt/axon/libaxon_pjrt.so"
    if "antenv.axon_hooks" not in _sys.modules:
        try:
            lib2 = ctypes.CDLL(so)
            lib2.axon_start_nrt_profile.argtypes = [
                ctypes.POINTER(ctypes.c_int64), ctypes.c_size_t]
            lib2.axon_start_nrt_profile.restype = ctypes.c_int64
            lib2.axon_stop_nrt_profile.argtypes = [ctypes.c_char_p]
            lib2.axon_stop_nrt_profile.restype = ctypes.c_int64

            @contextlib.contextmanager
            def _hook(output_dir, device_ids):
                import jax
                jax.devices()
                if device_ids:
                    ids_arr = (ctypes.c_int64 * len(device_ids))(*device_ids)
                    rc = lib2.axon_start_nrt_profile(ids_arr, len(device_ids))
                else:
                    rc = lib2.axon_start_nrt_profile(None, 0)
                if rc != 0:
                    raise RuntimeError(f"axon_start_nrt_profile rc={rc}")
                try:
                    yield
                finally:
                    lib2.axon_stop_nrt_profile(str(output_dir).encode())

            mod = _types.ModuleType("antenv.axon_hooks")
            mod.get_axon_ntff_profile_hook = lambda: _hook
            mod.set_axon_ntff_profile_hook = lambda h: None
            _sys.modules["antenv.axon_hooks"] = mod
        except OSError:
            pass
    import concourse.bass_utils as bass_utils
    bass_utils.upload_artifacts = lambda tmpdir: tmpdir

    nc, dev, B_shard = _prep_and_build(inputs)
    maps = _in_maps(inputs, dev, B_shard)
    return bass_utils.run_bass_kernel_spmd(
        nc, maps, core_ids=list(range(N_CORES)), trace=True, trace_cores=[0]
    )


# revision 7
# speedup vs baseline: 1.8188x; 1.0376x over previous
"""ARGenerator TRN2 kernel.

Math (per batch row b):
  h1 = relu(x @ W1.T + b1); h2 = relu(h1 @ W2.T + b2)
  mlp = tanh(h2 @ W3.T + b3)
  ar[t] = noise[t] + sum_i c[i] * ar[t-1-i]  (zero-init, t >= 7; 0 for t < 7)
  out = mlp + ar

The AR recurrence is linear time-invariant -> ar = conv(noise_masked, h)
with h the (geometrically decaying) impulse response, truncated at
L = nb*128 taps.  The conv becomes nb banded 128x128 Toeplitz matmuls
per output time-tile, fully parallel over time.

Layout strategy (pure data parallel over 8 cores, B_shard = 1024):
  Everything runs in bf16 (tolerance is 2e-2; bf16 keeps us ~5e-3) and
  in TRANSPOSED activation layout [feature/time on partitions, batch on
  free dim].  The host pre-transposes x and noise shards to [S, B_shard]
  bf16, so the device does plain contiguous DMA loads and zero PE
  transposes.  The output is produced transposed [S_OUT, B_shard] bf16
  (which lets the b3 bias + tanh fuse into one ACT instruction with a
  per-partition bias) and the host transposes it back.

Per-core phases:
  1. L1: psum[h, b] += W1l_k.T @ xT_k over 32 k-tiles; ACT relu+b1.
  2. L2: one matmul + ACT relu+b2 -> h2T [128, 1024] bf16.
  3. For each t-tile m (32):
       psm = W3l[:, m].T @ h2T          (128x1024, via 2 512-halves)
       psc = sum_j Hb_j.T @ nT[m+j]     (banded conv, nb matmuls/half)
       th  = tanh(psm + b3[m])          (ACT, per-partition bias)
       outT_m = th + psc  (DVE) -> bf16 -> DMA store.
DMA per core: 1MB W1 + 1MB W3 + 8MB xT + 8MB nT + 8MB out ~ 26MB.
"""

import numpy as np
import ml_dtypes

import concourse.bass as bass
import concourse.tile as tile
import concourse.mybir as mybir
from concourse import bacc

F32 = mybir.dt.float32
BF16 = mybir.dt.bfloat16
BF16_NP = ml_dtypes.bfloat16


def impulse_response(c, s_out, tail_tol=1e-4):
    """Return (h, nb) with nb*128 taps covering the response to tail_tol."""
    AR = len(c)
    c = np.asarray(c, np.float64)
    h = np.zeros(s_out, np.float64)
    h[0] = 1.0
    for j in range(1, s_out):
        acc = 0.0
        for i in range(AR):
            if j - 1 - i >= 0:
                acc += c[i] * h[j - 1 - i]
        h[j] = acc
    L = 128
    while L < s_out and np.abs(h[L:]).sum() > tail_tol:
        L += 128
    # nb = number of 128-wide band blocks per output tile: the in-tile block
    # (j=0) plus one per preceding input tile the L-tap history reaches into.
    return h, L // 128 + 1


def band_blocks(h, nb):
    """Hb [128, nb*128]: block jj (for input-tile offset j = jj - (nb-1))
    has Hb[k_rel, t_rel] = h[t_rel - k_rel - 128*j] (0 <= lag < (nb-1)*128)."""
    L = (nb - 1) * 128
    a = np.arange(128)[:, None]   # k_rel
    b = np.arange(128)[None, :]   # t_rel
    blocks = []
    for jj in range(nb):
        j = jj - (nb - 1)
        lag = b - a - 128 * j
        m = (lag >= 0) & (lag < L)
        blk = np.where(m, np.take(np.pad(h[:L], (0, 1)), np.clip(lag, 0, L)), 0.0)
        blocks.append(blk)
    return np.concatenate(blocks, axis=1)


def host_prepare(W1, b1, W2, b2, W3, b3, ar_coef, S_IN, S_OUT, H):
    """Small device tensors in exactly the SBUF layout used, bf16."""
    n_s = S_IN // 128
    # W1l[p, k*H + h] = W1[h, k*128 + p]  (lhsT tiles for layer 1)
    W1l = np.ascontiguousarray(
        W1.reshape(H, n_s, 128).transpose(2, 1, 0).reshape(128, n_s * H)
    )
    h, nb = impulse_response(ar_coef, S_OUT)
    return {
        "W1l": W1l.astype(BF16_NP),
        "W2l": np.ascontiguousarray(W2.T).astype(BF16_NP),   # [H_in, H_out]
        "W3l": np.ascontiguousarray(W3.T).astype(BF16_NP),   # [H, S_OUT]
        "b1c": np.ascontiguousarray(b1.reshape(H, 1), dtype=np.float32),
        "b2c": np.ascontiguousarray(b2.reshape(H, 1), dtype=np.float32),
        "b3m": np.ascontiguousarray(b3.reshape(S_OUT // 128, 128).T,
                                    dtype=np.float32),       # [128, n_t]
        "Hb": band_blocks(h, nb).astype(BF16_NP),
    }, nb


def build_kernel(B_shard, S_IN, S_OUT, H, nb):
    P = 128
    HF = 512                      # matmul moving width (one PSUM bank)
    assert H == P and B_shard % HF == 0
    n_s = S_IN // P               # 32 input k-tiles
    n_t = S_OUT // P              # 32 output t-tiles
    n_h = B_shard // HF           # batch halves

    nc = bacc.Bacc(trn_type="TRN2", target_bir_lowering=False, debug=False)

    xT_d = nc.dram_tensor("xT", [S_IN, B_shard], BF16, kind="ExternalInput").ap()
    nT_d = nc.dram_tensor("nT", [S_OUT, B_shard], BF16, kind="ExternalInput").ap()
    W1_d = nc.dram_tensor("W1l", [P, n_s * H], BF16, kind="ExternalInput").ap()
    W2_d = nc.dram_tensor("W2l", [H, H], BF16, kind="ExternalInput").ap()
    W3_d = nc.dram_tensor("W3l", [H, S_OUT], BF16, kind="ExternalInput").ap()
    b1_d = nc.dram_tensor("b1c", [H, 1], F32, kind="ExternalInput").ap()
    b2_d = nc.dram_tensor("b2c", [H, 1], F32, kind="ExternalInput").ap()
    b3_d = nc.dram_tensor("b3m", [P, n_t], F32, kind="ExternalInput").ap()
    Hb_d = nc.dram_tensor("Hb", [P, nb * P], BF16, kind="ExternalInput").ap()
    out_d = nc.dram_tensor("outT", [S_OUT, B_shard], BF16,
                           kind="ExternalOutput").ap()

    with tile.TileContext(nc) as tc:
        with tc.tile_pool(name="const", bufs=1) as cpool:
            # sync queue: W1 first (needed first), then the x tiles below.
            W1s = cpool.tile([P, n_s * H], BF16, tag="w1")
            nc.sync.dma_start(W1s[:], W1_d[:])
            # scalar queue: everything needed later.
            W2s = cpool.tile([H, H], BF16, tag="w2")
            nc.scalar.dma_start(W2s[:], W2_d[:])
            b1s = cpool.tile([H, 1], F32, tag="b1")
            nc.scalar.dma_start(b1s[:], b1_d[:])
            b2s = cpool.tile([H, 1], F32, tag="b2")
            nc.scalar.dma_start(b2s[:], b2_d[:])
            W3s = cpool.tile([H, S_OUT], BF16, tag="w3")
            nc.scalar.dma_start(W3s[:], W3_d[:])
            b3s = cpool.tile([P, n_t], F32, tag="b3")
            nc.scalar.dma_start(b3s[:], b3_d[:])
            Hbs = cpool.tile([P, nb * P], BF16, tag="hb")
            nc.scalar.dma_start(Hbs[:], Hb_d[:])

            with (
                tc.tile_pool(name="warm", bufs=1) as wpool,
                tc.tile_pool(name="xT", bufs=n_s) as xTp,
                tc.tile_pool(name="nT", bufs=10) as nTp,
                tc.tile_pool(name="act", bufs=2) as actp,
                tc.tile_pool(name="th", bufs=3) as thp,
                tc.tile_pool(name="outT", bufs=4) as outp,
                tc.tile_pool(name="ps", bufs=4, space="PSUM") as psp,
            ):
                # ---- PE warm-up: the HAM clock gate defaults the PE array to
                # 1.2 GHz and only releases to 2.4 GHz after ~3.4us of
                # sustained matmul activity.  The first real matmul cannot
                # start until W1+x arrive (~10us: framework init + DMA), so
                # keep the PE busy on a zeroed scratch tile until then --
                # otherwise the whole layer-1 phase runs at half clock.
                wsrc = wpool.tile([P, 4 * P], BF16, tag="wsrc")
                nc.vector.memset(wsrc[:], 0.0)
                wsnk = wpool.tile([P, 4], F32, tag="wsnk")
                psw = psp.tile([P, B_shard], F32, tag="ps", name="psw")
                for i in range(40):
                    nc.tensor.matmul(psw[:, :4 * P], wsrc[:, :P], wsrc[:])
                nc.vector.tensor_copy(wsnk[:], psw[:, :4])

                # ---- load all 32 xT tiles (resident; both halves use them)
                xts = []
                for k in range(n_s):
                    t = xTp.tile([P, B_shard], BF16, tag="xt", name=f"xt{k}")
                    nc.sync.dma_start(t[:], xT_d[k * P:(k + 1) * P, :])
                    xts.append(t)
                # ---- noise tiles stream behind on the same queue
                nts = [None] * n_t

                def load_nt(m):
                    nts[m] = nTp.tile([P, B_shard], BF16, tag="nt",
                                      name=f"nt{m}")
                    nc.sync.dma_start(nts[m][:], nT_d[m * P:(m + 1) * P, :])

                for m0 in range(8):
                    load_nt(m0)

                # ---- layer 1: psh1[h, b] = sum_k W1l_k.T @ xT_k
                psh1 = psp.tile([H, B_shard], F32, tag="ps", name="psh1")
                for hh in range(n_h):
                    sl = slice(hh * HF, (hh + 1) * HF)
                    for k in range(n_s):
                        nc.tensor.matmul(
                            psh1[:, sl],
                            W1s[:, k * H:(k + 1) * H],
                            xts[k][:, sl],
                            start=(k == 0),
                            stop=(k == n_s - 1),
                        )
                h1T = actp.tile([H, B_shard], BF16, tag="h1T")
                nc.scalar.activation(
                    h1T[:], psh1[:], mybir.ActivationFunctionType.Relu,
                    bias=b1s[:],
                )
                # ---- layer 2
                psh2 = psp.tile([H, B_shard], F32, tag="ps", name="psh2")
                for hh in range(n_h):
                    sl = slice(hh * HF, (hh + 1) * HF)
                    nc.tensor.matmul(psh2[:, sl], W2s[:], h1T[:, sl])
                h2T = actp.tile([H, B_shard], BF16, tag="h2T")
                nc.scalar.activation(
                    h2T[:], psh2[:], mybir.ActivationFunctionType.Relu,
                    bias=b2s[:],
                )

                # ---- per t-tile: L3 matmul + banded conv + tanh/bias + add
                for m in range(n_t):
                    if m + 8 < n_t:
                        load_nt(m + 8)
                    psc = psp.tile([P, B_shard], F32, tag="ps", name=f"psc{m}")
                    jlist = [j for j in range(-(nb - 1), 1) if m + j >= 0]
                    for hh in range(n_h):
                        sl = slice(hh * HF, (hh + 1) * HF)
                        for i, j in enumerate(jlist):
                            jj = j + nb - 1
                            nc.tensor.matmul(
                                psc[:, sl],
                                Hbs[:, jj * P:(jj + 1) * P],
                                nts[m + j][:, sl],
                                start=(i == 0),
                                stop=(i == len(jlist) - 1),
                            )
                    psm = psp.tile([P, B_shard], F32, tag="ps", name=f"psm{m}")
                    for hh in range(n_h):
                        sl = slice(hh * HF, (hh + 1) * HF)
                        nc.tensor.matmul(
                            psm[:, sl],
                            W3s[:, m * P:(m + 1) * P],
                            h2T[:, sl],
                        )
                    th = thp.tile([P, B_shard], F32, tag="th")
                    nc.scalar.activation(
                        th[:], psm[:], mybir.ActivationFunctionType.Tanh,
                        bias=b3s[:, m:m + 1],
                    )
                    ot = outp.tile([P, B_shard], BF16, tag="ot", name=f"ot{m}")
                    nc.vector.tensor_add(ot[:], th[:], psc[:])
                    # store triggers cost ~600ns of engine-queue time each;
                    # gpsimd is otherwise idle, scalar must keep doing tanh.
                    nc.gpsimd.dma_start(out_d[m * P:(m + 1) * P, :], ot[:])

    nc.compile()
    return nc


# ---------------------------------------------------------------------------
# Self-contained kernel() entry point (the graded contract).
# ---------------------------------------------------------------------------

N_CORES = 8
_B, _S_IN, _S_OUT, _H, _AR = 8192, 4096, 4096, 128, 7

_CACHE = {}


def _prep_and_build(inputs):
    dev, nb = host_prepare(
        np.asarray(inputs["W1"], np.float32), np.asarray(inputs["b1"], np.float32),
        np.asarray(inputs["W2"], np.float32), np.asarray(inputs["b2"], np.float32),
        np.asarray(inputs["W3"], np.float32), np.asarray(inputs["b3"], np.float32),
        np.asarray(inputs["ar_coef"], np.float32),
        _S_IN, _S_OUT, _H,
    )
    B_total = inputs["x"].shape[0]
    B_shard = B_total // N_CORES
    key = (B_shard, nb)
    if key not in _CACHE:
        _CACHE[key] = build_kernel(B_shard, _S_IN, _S_OUT, _H, nb)
    return _CACHE[key], dev, B_shard


def _in_maps(inputs, dev, B_shard):
    x = np.asarray(inputs["x"], np.float32)
    noise_m = np.asarray(inputs["noise"], np.float32).copy()
    noise_m[:, :_AR] = 0.0
    maps = []
    for c in range(N_CORES):
        sl = slice(c * B_shard, (c + 1) * B_shard)
        m = {"xT": np.ascontiguousarray(x[sl].astype(BF16_NP).T),
             "nT": np.ascontiguousarray(noise_m[sl].astype(BF16_NP).T)}
        m.update(dev)
        maps.append(m)
    return maps


def kernel(**inputs):
    nc, dev, B_shard = _prep_and_build(inputs)
    maps = _in_maps(inputs, dev, B_shard)
    import concourse.bass_utils as bass_utils

    res = bass_utils.run_bass_kernel_spmd(
        nc, maps, core_ids=list(range(N_CORES)), trace=False
    )
    return np.concatenate(
        [np.asarray(res.results[c]["outT"]).T for c in range(N_CORES)], axis=0
    ).astype(np.float32)


def run_traced(inputs):
    """Profiled run (NTFF -> neuron-profile) for the local test harness."""
    import contextlib
    import ctypes
    import sys as _sys
    import types as _types

    so = "/opt/axon/libaxon_pjrt.so"
    if "antenv.axon_hooks" not in _sys.modules:
        try:
            lib2 = ctypes.CDLL(so)
            lib2.axon_start_nrt_profile.argtypes = [
                ctypes.POINTER(ctypes.c_int64), ctypes.c_size_t]
            lib2.axon_start_nrt_profile.restype = ctypes.c_int64
            lib2.axon_stop_nrt_profile.argtypes = [ctypes.c_char_p]
            lib2.axon_stop_nrt_profile.restype = ctypes.c_int64

            @contextlib.contextmanager
            def _hook(output_dir, device_ids):
                import jax
                jax.devices()
                if device_ids:
                    ids_arr = (ctypes.c_int64 * len(device_ids))(*device_ids)
                    rc = lib2.axon_start_nrt_profile(ids_arr, len(device_ids))
                else:
                    rc = lib2.axon_start_nrt_profile(None, 0)
                if rc != 0:
                    raise RuntimeError(f"axon_start_nrt_profile rc={rc}")
                try:
                    yield
                finally:
                    lib2.axon_stop_nrt_profile(str(output_dir).encode())

            mod = _types.ModuleType("antenv.axon_hooks")
            mod.get_axon_ntff_profile_hook = lambda: _hook
            mod.set_axon_ntff_profile_hook = lambda h: None
            _sys.modules["antenv.axon_hooks"] = mod
        except OSError:
            pass
    import concourse.bass_utils as bass_utils
    bass_utils.upload_artifacts = lambda tmpdir: tmpdir

    nc, dev, B_shard = _prep_and_build(inputs)
    maps = _in_maps(inputs, dev, B_shard)
    return bass_utils.run_bass_kernel_spmd(
        nc, maps, core_ids=list(range(N_CORES)), trace=True, trace_cores=[0]
    )


# revision 9
# speedup vs baseline: 2.0410x; 1.1222x over previous
"""ARGenerator TRN2 kernel.

Math (per batch row b):
  h1 = relu(x @ W1.T + b1); h2 = relu(h1 @ W2.T + b2)
  mlp = tanh(h2 @ W3.T + b3)
  ar[t] = noise[t] + sum_i c[i] * ar[t-1-i]  (zero-init, t >= 7; 0 for t < 7)
  out = mlp + ar

The AR recurrence is linear time-invariant -> ar = conv(noise_masked, h)
with h the (geometrically decaying) impulse response, truncated at
L = nb*128 taps.  The conv becomes nb banded 128x128 Toeplitz matmuls
per output time-tile, fully parallel over time.

Layout strategy (pure data parallel over 8 cores, B_shard = 1024):
  Everything runs in bf16 (tolerance is 2e-2; bf16 keeps us ~5e-3) and
  in TRANSPOSED activation layout [feature/time on partitions, batch on
  free dim].  The host pre-transposes x and noise shards to [S, B_shard]
  bf16, so the device does plain contiguous DMA loads and zero PE
  transposes.  The output is produced transposed [S_OUT, B_shard] bf16
  (which lets the b3 bias + tanh fuse into one ACT instruction with a
  per-partition bias) and the host transposes it back.

Per-core phases:
  1. L1: psum[h, b] += W1l_k.T @ xT_k over 32 k-tiles; ACT relu+b1.
  2. L2: one matmul + ACT relu+b2 -> h2T [128, 1024] bf16.
  3. For each t-tile m (32):
       psm = W3l[:, m].T @ h2T          (128x1024, via 2 512-halves)
       psc = sum_j Hb_j.T @ nT[m+j]     (banded conv, nb matmuls/half)
       th  = tanh(psm + b3[m])          (ACT, per-partition bias)
       outT_m = th + psc  (DVE) -> bf16 -> DMA store.
DMA per core: 1MB W1 + 1MB W3 + 8MB xT + 8MB nT + 8MB out ~ 26MB.
"""

import numpy as np
import ml_dtypes

import concourse.bass as bass
import concourse.tile as tile
import concourse.mybir as mybir
from concourse import bacc

F32 = mybir.dt.float32
BF16 = mybir.dt.bfloat16
BF16_NP = ml_dtypes.bfloat16


def impulse_response(c, s_out, tail_tol=1e-4):
    """Return (h, nb) with nb*128 taps covering the response to tail_tol."""
    AR = len(c)
    c = np.asarray(c, np.float64)
    h = np.zeros(s_out, np.float64)
    h[0] = 1.0
    for j in range(1, s_out):
        acc = 0.0
        for i in range(AR):
            if j - 1 - i >= 0:
                acc += c[i] * h[j - 1 - i]
        h[j] = acc
    L = 128
    while L < s_out and np.abs(h[L:]).sum() > tail_tol:
        L += 128
    # nb = number of 128-wide band blocks per output tile: the in-tile block
    # (j=0) plus one per preceding input tile the L-tap history reaches into.
    return h, L // 128 + 1


def band_blocks(h, nb):
    """Hb [128, nb*128]: block jj (for input-tile offset j = jj - (nb-1))
    has Hb[k_rel, t_rel] = h[t_rel - k_rel - 128*j] (0 <= lag < (nb-1)*128)."""
    L = (nb - 1) * 128
    a = np.arange(128)[:, None]   # k_rel
    b = np.arange(128)[None, :]   # t_rel
    blocks = []
    for jj in range(nb):
        j = jj - (nb - 1)
        lag = b - a - 128 * j
        m = (lag >= 0) & (lag < L)
        blk = np.where(m, np.take(np.pad(h[:L], (0, 1)), np.clip(lag, 0, L)), 0.0)
        blocks.append(blk)
    return np.concatenate(blocks, axis=1)


def host_prepare(W1, b1, W2, b2, W3, b3, ar_coef, S_IN, S_OUT, H):
    """Small device tensors in exactly the SBUF layout used, bf16."""
    n_s = S_IN // 128
    # W1l[p, k*H + h] = W1[h, k*128 + p]  (lhsT tiles for layer 1)
    W1l = np.ascontiguousarray(
        W1.reshape(H, n_s, 128).transpose(2, 1, 0).reshape(128, n_s * H)
    )
    h, nb = impulse_response(ar_coef, S_OUT)
    return {
        "W1l": W1l.astype(BF16_NP),
        "W2l": np.ascontiguousarray(W2.T).astype(BF16_NP),   # [H_in, H_out]
        "W3l": np.ascontiguousarray(W3.T).astype(BF16_NP),   # [H, S_OUT]
        "b1c": np.ascontiguousarray(b1.reshape(H, 1), dtype=np.float32),
        "b2c": np.ascontiguousarray(b2.reshape(H, 1), dtype=np.float32),
        "b3m": np.ascontiguousarray(b3.reshape(S_OUT // 128, 128).T,
                                    dtype=np.float32),       # [128, n_t]
        "Hb": band_blocks(h, nb).astype(BF16_NP),
    }, nb


def build_kernel(B_shard, S_IN, S_OUT, H, nb):
    P = 128
    HF = 512                      # matmul moving width (one PSUM bank)
    assert H == P and B_shard % HF == 0
    n_s = S_IN // P               # 32 input k-tiles
    n_t = S_OUT // P              # 32 output t-tiles
    n_h = B_shard // HF           # batch halves

    nc = bacc.Bacc(trn_type="TRN2", target_bir_lowering=False, debug=False)

    xT_d = nc.dram_tensor("xT", [S_IN, B_shard], BF16, kind="ExternalInput").ap()
    nT_d = nc.dram_tensor("nT", [S_OUT, B_shard], BF16, kind="ExternalInput").ap()
    W1_d = nc.dram_tensor("W1l", [P, n_s * H], BF16, kind="ExternalInput").ap()
    W2_d = nc.dram_tensor("W2l", [H, H], BF16, kind="ExternalInput").ap()
    W3_d = nc.dram_tensor("W3l", [H, S_OUT], BF16, kind="ExternalInput").ap()
    b1_d = nc.dram_tensor("b1c", [H, 1], F32, kind="ExternalInput").ap()
    b2_d = nc.dram_tensor("b2c", [H, 1], F32, kind="ExternalInput").ap()
    b3_d = nc.dram_tensor("b3m", [P, n_t], F32, kind="ExternalInput").ap()
    Hb_d = nc.dram_tensor("Hb", [P, nb * P], BF16, kind="ExternalInput").ap()
    out_d = nc.dram_tensor("outT", [S_OUT, B_shard], BF16,
                           kind="ExternalOutput").ap()

    with tile.TileContext(nc) as tc:
        with tc.tile_pool(name="const", bufs=1) as cpool:
            # sync queue: W1 first (needed first), then the x tiles below.
            W1s = cpool.tile([P, n_s * H], BF16, tag="w1")
            nc.sync.dma_start(W1s[:], W1_d[:])
            # scalar queue: everything needed later.
            W2s = cpool.tile([H, H], BF16, tag="w2")
            nc.scalar.dma_start(W2s[:], W2_d[:])
            b1s = cpool.tile([H, 1], F32, tag="b1")
            nc.scalar.dma_start(b1s[:], b1_d[:])
            b2s = cpool.tile([H, 1], F32, tag="b2")
            nc.scalar.dma_start(b2s[:], b2_d[:])
            W3s = cpool.tile([H, S_OUT], BF16, tag="w3")
            nc.scalar.dma_start(W3s[:], W3_d[:])
            b3s = cpool.tile([P, n_t], F32, tag="b3")
            nc.scalar.dma_start(b3s[:], b3_d[:])
            Hbs = cpool.tile([P, nb * P], BF16, tag="hb")
            nc.scalar.dma_start(Hbs[:], Hb_d[:])

            with (
                tc.tile_pool(name="warm", bufs=1) as wpool,
                tc.tile_pool(name="xT", bufs=n_s // 4) as xTp,
                tc.tile_pool(name="nT", bufs=6) as nTp,
                tc.tile_pool(name="act", bufs=2) as actp,
                tc.tile_pool(name="th", bufs=3) as thp,
                tc.tile_pool(name="outT", bufs=4) as outp,
                tc.tile_pool(name="psA", bufs=2, space="PSUM") as psA,
                tc.tile_pool(name="psB", bufs=4, space="PSUM") as psB,
            ):
                # ---- PE warm-up: the HAM clock gate defaults the PE array to
                # 1.2 GHz and only releases to 2.4 GHz after ~3.4us of
                # sustained matmul activity.  The first real matmul cannot
                # start until W1+x arrive (~10us: framework init + DMA), so
                # keep the PE busy on a zeroed scratch tile until then --
                # otherwise the whole layer-1 phase runs at half clock.
                wsrc = wpool.tile([P, 4 * P], BF16, tag="wsrc")
                nc.vector.memset(wsrc[:], 0.0)
                wsnk = wpool.tile([P, 4], F32, tag="wsnk")
                psw = psB.tile([P, HF], F32, tag="ps", name="psw")
                for i in range(20):
                    nc.tensor.matmul(psw[:], wsrc[:, :P], wsrc[:])
                nc.vector.tensor_copy(wsnk[:], psw[:, :4])

                # ---- all 32 xT k-tiles, merged 4 per DMA (1MB each), the
                # 8 triggers alternating between the sync and scalar queues
                # so a single queue's trigger rate doesn't cap the wire.
                xts = []
                for g in range(n_s // 4):
                    t = xTp.tile([P, 4, B_shard], BF16, tag="xt", name=f"xt{g}")
                    src = xT_d[g * 4 * P:(g + 1) * 4 * P, :].rearrange(
                        "(blk p) f -> p blk f", p=P)
                    (nc.sync if g % 2 == 0 else nc.scalar).dma_start(t[:], src)
                    xts.append(t)

                def xt(k):
                    return xts[k // 4][:, k % 4, :]

                # ---- noise t-tiles, merged 2 per DMA, sync queue
                ntm = [None] * (n_t // 2)

                def load_nt2(g):
                    ntm[g] = nTp.tile([P, 2, B_shard], BF16, tag="nt",
                                      name=f"nt{g}")
                    src = nT_d[g * 2 * P:(g + 1) * 2 * P, :].rearrange(
                        "(blk p) f -> p blk f", p=P)
                    nc.sync.dma_start(ntm[g][:], src)

                def nt(m):
                    return ntm[m // 2][:, m % 2, :]

                for g0 in range(4):
                    load_nt2(g0)

                # ---- layer 1: psh1[h, b] = sum_k W1l_k.T @ xT_k
                psh1 = psA.tile([H, B_shard], F32, tag="psA", name="psh1")
                for hh in range(n_h):
                    sl = slice(hh * HF, (hh + 1) * HF)
                    for k in range(n_s):
                        nc.tensor.matmul(
                            psh1[:, sl],
                            W1s[:, k * H:(k + 1) * H],
                            xt(k)[:, sl],
                            start=(k == 0),
                            stop=(k == n_s - 1),
                        )
                h1T = actp.tile([H, B_shard], BF16, tag="h1T")
                nc.scalar.activation(
                    h1T[:], psh1[:], mybir.ActivationFunctionType.Relu,
                    bias=b1s[:],
                )
                # ---- layer 2
                psh2 = psA.tile([H, B_shard], F32, tag="psA", name="psh2")
                for hh in range(n_h):
                    sl = slice(hh * HF, (hh + 1) * HF)
                    nc.tensor.matmul(psh2[:, sl], W2s[:], h1T[:, sl])
                h2T = actp.tile([H, B_shard], BF16, tag="h2T")
                nc.scalar.activation(
                    h2T[:], psh2[:], mybir.ActivationFunctionType.Relu,
                    bias=b2s[:],
                )

                # ---- per t-tile: banded conv + L3 matmul + tanh/bias + add.
                # psc lives in single-bank halves so the DVE add releases
                # PSUM at half-tile granularity; psm is a 2-bank tile so the
                # tanh reads it in one full-width ACT instruction.
                for m in range(n_t):
                    if m % 2 == 0 and (m + 8) // 2 < n_t // 2:
                        load_nt2((m + 8) // 2)
                    jlist = [j for j in range(-(nb - 1), 1) if m + j >= 0]
                    pscs = []
                    for hh in range(n_h):
                        sl = slice(hh * HF, (hh + 1) * HF)
                        psc = psB.tile([P, HF], F32, tag="ps",
                                       name=f"psc{m}_{hh}")
                        pscs.append(psc)
                        for i, j in enumerate(jlist):
                            jj = j + nb - 1
                            nc.tensor.matmul(
                                psc[:],
                                Hbs[:, jj * P:(jj + 1) * P],
                                nt(m + j)[:, sl],
                                start=(i == 0),
                                stop=(i == len(jlist) - 1),
                            )
                    psm = psA.tile([P, B_shard], F32, tag="psA", name=f"psm{m}")
                    for hh in range(n_h):
                        sl = slice(hh * HF, (hh + 1) * HF)
                        nc.tensor.matmul(
                            psm[:, sl],
                            W3s[:, m * P:(m + 1) * P],
                            h2T[:, sl],
                        )
                    th = thp.tile([P, B_shard], F32, tag="th")
                    nc.scalar.activation(
                        th[:], psm[:], mybir.ActivationFunctionType.Tanh,
                        bias=b3s[:, m:m + 1],
                    )
                    ot = outp.tile([P, B_shard], BF16, tag="ot", name=f"ot{m}")
                    for hh in range(n_h):
                        sl = slice(hh * HF, (hh + 1) * HF)
                        # gpsimd cannot read PSUM, so the adds must all be DVE
                        nc.vector.tensor_add(ot[:, sl], th[:, sl], pscs[hh][:])
                    # stores on the (otherwise idle) gpsimd queue: on sync
                    # they would head-of-line-block the noise loads.
                    nc.gpsimd.dma_start(out_d[m * P:(m + 1) * P, :], ot[:])

    nc.compile()
    return nc


# ---------------------------------------------------------------------------
# Self-contained kernel() entry point (the graded contract).
# ---------------------------------------------------------------------------

N_CORES = 8
_B, _S_IN, _S_OUT, _H, _AR = 8192, 4096, 4096, 128, 7

_CACHE = {}


def _prep_and_build(inputs):
    dev, nb = host_prepare(
        np.asarray(inputs["W1"], np.float32), np.asarray(inputs["b1"], np.float32),
        np.asarray(inputs["W2"], np.float32), np.asarray(inputs["b2"], np.float32),
        np.asarray(inputs["W3"], np.float32), np.asarray(inputs["b3"], np.float32),
        np.asarray(inputs["ar_coef"], np.float32),
        _S_IN, _S_OUT, _H,
    )
    B_total = inputs["x"].shape[0]
    B_shard = B_total // N_CORES
    key = (B_shard, nb)
    if key not in _CACHE:
        _CACHE[key] = build_kernel(B_shard, _S_IN, _S_OUT, _H, nb)
    return _CACHE[key], dev, B_shard


def _in_maps(inputs, dev, B_shard):
    x = np.asarray(inputs["x"], np.float32)
    noise_m = np.asarray(inputs["noise"], np.float32).copy()
    noise_m[:, :_AR] = 0.0
    maps = []
    for c in range(N_CORES):
        sl = slice(c * B_shard, (c + 1) * B_shard)
        m = {"xT": np.ascontiguousarray(x[sl].astype(BF16_NP).T),
             "nT": np.ascontiguousarray(noise_m[sl].astype(BF16_NP).T)}
        m.update(dev)
        maps.append(m)
    return maps


def kernel(**inputs):
    nc, dev, B_shard = _prep_and_build(inputs)
    maps = _in_maps(inputs, dev, B_shard)
    import concourse.bass_utils as bass_utils

    res = bass_utils.run_bass_kernel_spmd(
        nc, maps, core_ids=list(range(N_CORES)), trace=False
    )
    return np.concatenate(
        [np.asarray(res.results[c]["outT"]).T for c in range(N_CORES)], axis=0
    ).astype(np.float32)


def run_traced(inputs):
    """Profiled run (NTFF -> neuron-profile) for the local test harness."""
    import contextlib
    import ctypes
    import sys as _sys
    import types as _types

    so = "/opt/axon/libaxon_pjrt.so"
    if "antenv.axon_hooks" not in _sys.modules:
        try:
            lib2 = ctypes.CDLL(so)
            lib2.axon_start_nrt_profile.argtypes = [
                ctypes.POINTER(ctypes.c_int64), ctypes.c_size_t]
            lib2.axon_start_nrt_profile.restype = ctypes.c_int64
            lib2.axon_stop_nrt_profile.argtypes = [ctypes.c_char_p]
            lib2.axon_stop_nrt_profile.restype = ctypes.c_int64

            @contextlib.contextmanager
            def _hook(output_dir, device_ids):
                import jax
                jax.devices()
                if device_ids:
                    ids_arr = (ctypes.c_int64 * len(device_ids))(*device_ids)
                    rc = lib2.axon_start_nrt_profile(ids_arr, len(device_ids))
                else:
                    rc = lib2.axon_start_nrt_profile(None, 0)
                if rc != 0:
                    raise RuntimeError(f"axon_start_nrt_profile rc={rc}")
                try:
                    yield
                finally:
                    lib2.axon_stop_nrt_profile(str(output_dir).encode())

            mod = _types.ModuleType("antenv.axon_hooks")
            mod.get_axon_ntff_profile_hook = lambda: _hook
            mod.set_axon_ntff_profile_hook = lambda h: None
            _sys.modules["antenv.axon_hooks"] = mod
        except OSError:
            pass
    import concourse.bass_utils as bass_utils
    bass_utils.upload_artifacts = lambda tmpdir: tmpdir

    nc, dev, B_shard = _prep_and_build(inputs)
    maps = _in_maps(inputs, dev, B_shard)
    return bass_utils.run_bass_kernel_spmd(
        nc, maps, core_ids=list(range(N_CORES)), trace=True, trace_cores=[0]
    )


# revision 11
# speedup vs baseline: 2.1158x; 1.0367x over previous
"""ARGenerator TRN2 kernel.

Math (per batch row b):
  h1 = relu(x @ W1.T + b1); h2 = relu(h1 @ W2.T + b2)
  mlp = tanh(h2 @ W3.T + b3)
  ar[t] = noise[t] + sum_i c[i] * ar[t-1-i]  (zero-init, t >= 7; 0 for t < 7)
  out = mlp + ar

The AR recurrence is linear time-invariant -> ar = conv(noise_masked, h)
with h the (geometrically decaying) impulse response, truncated at
(nb-1)*128 taps.  The conv becomes nb banded 128x128 Toeplitz matmuls
per output time-tile, fully parallel over time.

Layout strategy (pure data parallel over 8 cores, B_shard = 1024):
  Everything runs in bf16 (tolerance is 2e-2; bf16 keeps us ~5e-3) and
  in TRANSPOSED activation layout [feature/time on partitions, batch on
  free dim].  The host pre-transposes x and noise shards, so the device
  does plain contiguous DMA loads and zero PE transposes.  The output
  is produced transposed (which lets the b3 bias + tanh fuse into one
  ACT instruction with a per-partition bias) and the host transposes it
  back.

Pipeline: the batch shard is processed in 2 chunks of 512 so that
chunk B's input DMA and layer-1 matmuls overlap chunk A's t-loop
(the t-loop is wire-paced, leaving PE idle slack that exactly fits
L1(B)); chunk A's stores likewise overlap chunk B's loads.  Per-core
wire traffic: 1MB W1 + 1.1MB consts + 8MB xT + 8MB nT + 8MB out.

Per-chunk phases:
  1. L1: psh1[h, b] += W1l_k.T @ xT_k over 32 k-tiles; ACT relu+b1.
  2. L2: one matmul + ACT relu+b2 -> h2T [128, 512] bf16.
  3. For each t-tile m (32):
       psc = sum_j Hb_j.T @ nT[m+j]     (banded conv, nb matmuls)
       psm = W3l[:, m].T @ h2T          (one matmul)
       th  = tanh(psm + b3[m])          (ACT, per-partition bias)
       out = th + psc  (DVE, the only engine that can read PSUM)
       -> bf16, stores merged x2 on the gpsimd queue.
"""

import numpy as np
import ml_dtypes

import concourse.bass as bass
import concourse.tile as tile
import concourse.mybir as mybir
from concourse import bacc

F32 = mybir.dt.float32
BF16 = mybir.dt.bfloat16
BF16_NP = ml_dtypes.bfloat16


def impulse_response(c, s_out, tail_tol=1e-4):
    """Return (h, nb) with (nb-1)*128 taps covering the response."""
    AR = len(c)
    c = np.asarray(c, np.float64)
    h = np.zeros(s_out, np.float64)
    h[0] = 1.0
    for j in range(1, s_out):
        acc = 0.0
        for i in range(AR):
            if j - 1 - i >= 0:
                acc += c[i] * h[j - 1 - i]
        h[j] = acc
    L = 128
    while L < s_out and np.abs(h[L:]).sum() > tail_tol:
        L += 128
    # nb = number of 128-wide band blocks per output tile: the in-tile block
    # (j=0) plus one per preceding input tile the L-tap history reaches into.
    return h, L // 128 + 1


def band_blocks(h, nb):
    """Hb [128, nb*128]: block jj (for input-tile offset j = jj - (nb-1))
    has Hb[k_rel, t_rel] = h[t_rel - k_rel - 128*j] (0 <= lag < (nb-1)*128)."""
    L = (nb - 1) * 128
    a = np.arange(128)[:, None]   # k_rel
    b = np.arange(128)[None, :]   # t_rel
    blocks = []
    for jj in range(nb):
        j = jj - (nb - 1)
        lag = b - a - 128 * j
        m = (lag >= 0) & (lag < L)
        blk = np.where(m, np.take(np.pad(h[:L], (0, 1)), np.clip(lag, 0, L)), 0.0)
        blocks.append(blk)
    return np.concatenate(blocks, axis=1)


def host_prepare(W1, b1, W2, b2, W3, b3, ar_coef, S_IN, S_OUT, H):
    """Small device tensors in exactly the SBUF layout used, bf16."""
    n_s = S_IN // 128
    # W1l[p, k*H + h] = W1[h, k*128 + p]  (lhsT tiles for layer 1)
    W1l = np.ascontiguousarray(
        W1.reshape(H, n_s, 128).transpose(2, 1, 0).reshape(128, n_s * H)
    )
    h, nb = impulse_response(ar_coef, S_OUT)
    return {
        "W1l": W1l.astype(BF16_NP),
        "W2l": np.ascontiguousarray(W2.T).astype(BF16_NP),   # [H_in, H_out]
        "W3l": np.ascontiguousarray(W3.T).astype(BF16_NP),   # [H, S_OUT]
        "b1c": np.ascontiguousarray(b1.reshape(H, 1), dtype=np.float32),
        "b2c": np.ascontiguousarray(b2.reshape(H, 1), dtype=np.float32),
        "b3m": np.ascontiguousarray(b3.reshape(S_OUT // 128, 128).T,
                                    dtype=np.float32),       # [128, n_t]
        "Hb": band_blocks(h, nb).astype(BF16_NP),
    }, nb


def build_kernel(B_shard, S_IN, S_OUT, H, nb):
    P = 128
    NC = 2                        # batch chunks
    CW = B_shard // NC            # chunk width (free dim of every op)
    assert H == P and CW == 512
    n_s = S_IN // P               # 32 input k-tiles
    n_t = S_OUT // P              # 32 output t-tiles

    nc = bacc.Bacc(trn_type="TRN2", target_bir_lowering=False, debug=False)

    xT_d = nc.dram_tensor("xT", [NC, S_IN, CW], BF16, kind="ExternalInput").ap()
    nT_d = nc.dram_tensor("nT", [NC, S_OUT, CW], BF16, kind="ExternalInput").ap()
    W1_d = nc.dram_tensor("W1l", [P, n_s * H], BF16, kind="ExternalInput").ap()
    W2_d = nc.dram_tensor("W2l", [H, H], BF16, kind="ExternalInput").ap()
    W3_d = nc.dram_tensor("W3l", [H, S_OUT], BF16, kind="ExternalInput").ap()
    b1_d = nc.dram_tensor("b1c", [H, 1], F32, kind="ExternalInput").ap()
    b2_d = nc.dram_tensor("b2c", [H, 1], F32, kind="ExternalInput").ap()
    b3_d = nc.dram_tensor("b3m", [P, n_t], F32, kind="ExternalInput").ap()
    Hb_d = nc.dram_tensor("Hb", [P, nb * P], BF16, kind="ExternalInput").ap()
    out_d = nc.dram_tensor("outT", [NC, S_OUT, CW], BF16,
                           kind="ExternalOutput").ap()

    with tile.TileContext(nc) as tc:
        with tc.tile_pool(name="const", bufs=1) as cpool:
            # sync queue: W1 first (needed first), then x/noise tiles below.
            W1s = cpool.tile([P, n_s * H], BF16, tag="w1")
            nc.sync.dma_start(W1s[:], W1_d[:])
            # scalar queue: small consts, then it helps carry chunk-A x.
            W2s = cpool.tile([H, H], BF16, tag="w2")
            nc.scalar.dma_start(W2s[:], W2_d[:])
            b1s = cpool.tile([H, 1], F32, tag="b1")
            nc.scalar.dma_start(b1s[:], b1_d[:])
            b2s = cpool.tile([H, 1], F32, tag="b2")
            nc.scalar.dma_start(b2s[:], b2_d[:])
            b3s = cpool.tile([P, n_t], F32, tag="b3")
            nc.scalar.dma_start(b3s[:], b3_d[:])
            Hbs = cpool.tile([P, nb * P], BF16, tag="hb")
            nc.scalar.dma_start(Hbs[:], Hb_d[:])
            W3s = cpool.tile([H, S_OUT], BF16, tag="w3")
            nc.scalar.dma_start(W3s[:], W3_d[:])

            with (
                tc.tile_pool(name="warm", bufs=1) as wpool,
                tc.tile_pool(name="xT", bufs=n_s // 4) as xTp,
                tc.tile_pool(name="nT", bufs=4) as nTp,
                tc.tile_pool(name="act", bufs=2) as actp,
                tc.tile_pool(name="th", bufs=4) as thp,
                tc.tile_pool(name="outT", bufs=4) as outp,
                tc.tile_pool(name="psA", bufs=3, space="PSUM") as psA,
                tc.tile_pool(name="psB", bufs=4, space="PSUM") as psB,
            ):
                # ---- PE warm-up: the HAM clock gate defaults the PE array
                # to 1.2 GHz and only releases 2.4 GHz after ~3.4us of
                # sustained matmul activity; it re-throttles after ~3.4us
                # idle.  The first real matmul cannot start until W1+x
                # arrive (~13us: framework init + DMA), so keep the PE busy
                # on a zeroed scratch tile until then -- otherwise the whole
                # layer-1 phase runs at half clock.
                wsrc = wpool.tile([P, 4 * P], BF16, tag="wsrc")
                nc.vector.memset(wsrc[:], 0.0)
                wsnk = wpool.tile([P, 4], F32, tag="wsnk")
                psw = psB.tile([P, CW], F32, tag="ps", name="psw")
                for i in range(22):
                    nc.tensor.matmul(psw[:], wsrc[:, :P], wsrc[:])
                nc.vector.tensor_copy(wsnk[:], psw[:, :4])

                # ---- chunked input loads: 4 k-tiles per DMA (512KB)
                xts = {}

                def load_x4(c, g, eng):
                    t = xTp.tile([P, 4, CW], BF16, tag="xt", name=f"xt{c}_{g}")
                    src = xT_d[c, g * 4 * P:(g + 1) * 4 * P, :].rearrange(
                        "(blk p) f -> p blk f", p=P)
                    eng.dma_start(t[:], src)
                    xts[c, g] = t

                def xt(c, k):
                    return xts[c, k // 4][:, k % 4, :]

                ntm = {}

                def load_n4(c, g):
                    t = nTp.tile([P, 4, CW], BF16, tag="nt", name=f"nt{c}_{g}")
                    src = nT_d[c, g * 4 * P:(g + 1) * 4 * P, :].rearrange(
                        "(blk p) f -> p blk f", p=P)
                    nc.sync.dma_start(t[:], src)
                    ntm[c, g] = t

                def nt(c, m):
                    return ntm[c, m // 4][:, m % 4, :]

                # chunk A x: alternate sync/scalar so one queue's trigger
                # rate doesn't cap the wire.
                for g in range(n_s // 4):
                    load_x4(0, g, nc.sync if g % 2 == 0 else nc.scalar)
                # chunk A noise + chunk B x, interleaved on sync: both are
                # consumed at ~1 tile per t-loop-A step.
                load_n4(0, 0)
                for g in range(n_s // 4):
                    load_x4(1, g, nc.sync)
                    if g + 1 < 8:
                        load_n4(0, g + 1)

                def layer12(c, psh1):
                    h1T = actp.tile([H, CW], BF16, tag="act", name=f"h1T{c}")
                    nc.scalar.activation(
                        h1T[:], psh1[:], mybir.ActivationFunctionType.Relu,
                        bias=b1s[:],
                    )
                    psh2 = psA.tile([H, CW], F32, tag="psA", name=f"psh2{c}")
                    nc.tensor.matmul(psh2[:], W2s[:], h1T[:])
                    h2T = actp.tile([H, CW], BF16, tag="act", name=f"h2T{c}")
                    nc.scalar.activation(
                        h2T[:], psh2[:], mybir.ActivationFunctionType.Relu,
                        bias=b2s[:],
                    )
                    return h2T

                # ---- L1(A) as one dense run; L1(B) is interleaved into
                # t-loop-A below (1 matmul per t-tile) to use the PE slack
                # while that loop is wire-paced.
                psh1a = psA.tile([H, CW], F32, tag="psA", name="psh1a")
                for k in range(n_s):
                    nc.tensor.matmul(
                        psh1a[:], W1s[:, k * H:(k + 1) * H], xt(0, k),
                        start=(k == 0), stop=(k == n_s - 1),
                    )
                h2Ta = layer12(0, psh1a)
                psh1b = psA.tile([H, CW], F32, tag="psA", name="psh1b")

                def t_loop(c, h2T, extra=None):
                    for m in range(n_t):
                        if m % 4 == 0:
                            g = (m + 8) // 4
                            if c == 0:
                                # chunk-A groups were all queued up front
                                # (pool backpressure self-paces them)
                                if g >= n_t // 4:
                                    load_n4(1, g - n_t // 4)
                            elif g < n_t // 4:
                                load_n4(c, g)
                        jlist = [j for j in range(-(nb - 1), 1) if m + j >= 0]
                        psc = psB.tile([P, CW], F32, tag="ps",
                                       name=f"psc{c}_{m}")
                        for i, j in enumerate(jlist):
                            jj = j + nb - 1
                            nc.tensor.matmul(
                                psc[:], Hbs[:, jj * P:(jj + 1) * P],
                                nt(c, m + j),
                                start=(i == 0), stop=(i == len(jlist) - 1),
                            )
                        psm = psA.tile([P, CW], F32, tag="psA",
                                       name=f"psm{c}_{m}")
                        nc.tensor.matmul(
                            psm[:], W3s[:, m * P:(m + 1) * P], h2T[:],
                        )
                        if extra is not None:
                            extra(m)
                        th = thp.tile([P, CW], F32, tag="th")
                        nc.scalar.activation(
                            th[:], psm[:], mybir.ActivationFunctionType.Tanh,
                            bias=b3s[:, m:m + 1],
                        )
                        if m % 2 == 0:
                            ot = outp.tile([P, 2, CW], BF16, tag="ot",
                                           name=f"ot{c}_{m // 2}")
                        nc.vector.tensor_add(ot[:, m % 2, :], th[:], psc[:])
                        if m % 2 == 1:
                            dst = out_d[c, (m - 1) * P:(m + 1) * P, :].rearrange(
                                "(blk p) f -> p blk f", p=P)
                            nc.gpsimd.dma_start(dst, ot[:])

                def l1b_step(m):
                    nc.tensor.matmul(
                        psh1b[:], W1s[:, m * H:(m + 1) * H], xt(1, m),
                        start=(m == 0), stop=(m == n_s - 1),
                    )

                t_loop(0, h2Ta, extra=l1b_step)
                h2Tb = layer12(1, psh1b)
                t_loop(1, h2Tb)

    nc.compile()
    return nc


# ---------------------------------------------------------------------------
# Self-contained kernel() entry point (the graded contract).
# ---------------------------------------------------------------------------

N_CORES = 8
_B, _S_IN, _S_OUT, _H, _AR = 8192, 4096, 4096, 128, 7
_CW = 512

_CACHE = {}


def _prep_and_build(inputs):
    dev, nb = host_prepare(
        np.asarray(inputs["W1"], np.float32), np.asarray(inputs["b1"], np.float32),
        np.asarray(inputs["W2"], np.float32), np.asarray(inputs["b2"], np.float32),
        np.asarray(inputs["W3"], np.float32), np.asarray(inputs["b3"], np.float32),
        np.asarray(inputs["ar_coef"], np.float32),
        _S_IN, _S_OUT, _H,
    )
    B_total = inputs["x"].shape[0]
    B_shard = B_total // N_CORES
    key = (B_shard, nb)
    if key not in _CACHE:
        _CACHE[key] = build_kernel(B_shard, _S_IN, _S_OUT, _H, nb)
    return _CACHE[key], dev, B_shard


def _chunked_T(a, B_shard):
    """[B_shard, S] fp32 -> [2, S, B_shard//2] bf16 (transposed chunks)."""
    aT = a.astype(BF16_NP).T                      # [S, B_shard]
    cw = B_shard // 2
    return np.ascontiguousarray(
        np.stack([aT[:, :cw], aT[:, cw:]]))


def _in_maps(inputs, dev, B_shard):
    x = np.asarray(inputs["x"], np.float32)
    noise_m = np.asarray(inputs["noise"], np.float32).copy()
    noise_m[:, :_AR] = 0.0
    maps = []
    for c in range(N_CORES):
        sl = slice(c * B_shard, (c + 1) * B_shard)
        m = {"xT": _chunked_T(x[sl], B_shard),
             "nT": _chunked_T(noise_m[sl], B_shard)}
        m.update(dev)
        maps.append(m)
    return maps


def kernel(**inputs):
    nc, dev, B_shard = _prep_and_build(inputs)
    maps = _in_maps(inputs, dev, B_shard)
    import concourse.bass_utils as bass_utils

    res = bass_utils.run_bass_kernel_spmd(
        nc, maps, core_ids=list(range(N_CORES)), trace=False
    )
    shards = []
    for c in range(N_CORES):
        o = np.asarray(res.results[c]["outT"])    # [2, S_OUT, CW] bf16
        shards.append(np.concatenate([o[0].T, o[1].T], axis=0))
    return np.concatenate(shards, axis=0).astype(np.float32)


def run_traced(inputs):
    """Profiled run (NTFF -> neuron-profile) for the local test harness."""
    import contextlib
    import ctypes
    import sys as _sys
    import types as _types

    so = "/opt/axon/libaxon_pjrt.so"
    if "antenv.axon_hooks" not in _sys.modules:
        try:
            lib2 = ctypes.CDLL(so)
            lib2.axon_start_nrt_profile.argtypes = [
                ctypes.POINTER(ctypes.c_int64), ctypes.c_size_t]
            lib2.axon_start_nrt_profile.restype = ctypes.c_int64
            lib2.axon_stop_nrt_profile.argtypes = [ctypes.c_char_p]
            lib2.axon_stop_nrt_profile.restype = ctypes.c_int64

            @contextlib.contextmanager
            def _hook(output_dir, device_ids):
                import jax
                jax.devices()
                if device_ids:
                    ids_arr = (ctypes.c_int64 * len(device_ids))(*device_ids)
                    rc = lib2.axon_start_nrt_profile(ids_arr, len(device_ids))
                else:
                    rc = lib2.axon_start_nrt_profile(None, 0)
                if rc != 0:
                    raise RuntimeError(f"axon_start_nrt_profile rc={rc}")
                try:
                    yield
                finally:
                    lib2.axon_stop_nrt_profile(str(output_dir).encode())

            mod = _types.ModuleType("antenv.axon_hooks")
            mod.get_axon_ntff_profile_hook = lambda: _hook
            mod.set_axon_ntff_profile_hook = lambda h: None
            _sys.modules["antenv.axon_hooks"] = mod
        except OSError:
            pass
    import concourse.bass_utils as bass_utils
    bass_utils.upload_artifacts = lambda tmpdir: tmpdir

    nc, dev, B_shard = _prep_and_build(inputs)
    maps = _in_maps(inputs, dev, B_shard)
    return bass_utils.run_bass_kernel_spmd(
        nc, maps, core_ids=list(range(N_CORES)), trace=True, trace_cores=[0]
    )


# revision 12
# speedup vs baseline: 2.1997x; 1.0397x over previous
"""ARGenerator TRN2 kernel.

Math (per batch row b):
  h1 = relu(x @ W1.T + b1); h2 = relu(h1 @ W2.T + b2)
  mlp = tanh(h2 @ W3.T + b3)
  ar[t] = noise[t] + sum_i c[i] * ar[t-1-i]  (zero-init, t >= 7; 0 for t < 7)
  out = mlp + ar

The AR recurrence is linear time-invariant -> ar = conv(noise_masked, h)
with h the (geometrically decaying) impulse response, truncated at
(nb-1)*128 taps.  The conv becomes nb banded 128x128 Toeplitz matmuls
per output time-tile, fully parallel over time.

Layout strategy (pure data parallel over 8 cores, B_shard = 1024):
  Everything runs in bf16 (tolerance is 2e-2; bf16 keeps us ~5e-3) and
  in TRANSPOSED activation layout [feature/time on partitions, batch on
  free dim].  The host pre-transposes x and noise shards, so the device
  does plain contiguous DMA loads and zero PE transposes.  The output
  is produced transposed (which lets the b3 bias + tanh fuse into one
  ACT instruction with a per-partition bias) and the host transposes it
  back.

Pipeline: the batch shard is processed in 2 chunks of 512 so that
chunk B's input DMA and layer-1 matmuls overlap chunk A's t-loop
(the t-loop is wire-paced, leaving PE idle slack that exactly fits
L1(B)); chunk A's stores likewise overlap chunk B's loads.  Per-core
wire traffic: 1MB W1 + 1.1MB consts + 8MB xT + 8MB nT + 8MB out.

Per-chunk phases:
  1. L1: psh1[h, b] += W1l_k.T @ xT_k over 32 k-tiles; ACT relu+b1.
  2. L2: one matmul + ACT relu+b2 -> h2T [128, 512] bf16.
  3. For each t-tile m (32):
       psc = sum_j Hb_j.T @ nT[m+j]     (banded conv, nb matmuls)
       psm = W3l[:, m].T @ h2T          (one matmul)
       th  = tanh(psm + b3[m])          (ACT, per-partition bias)
       out = th + psc  (DVE, the only engine that can read PSUM)
       -> bf16, stores merged x2 on the gpsimd queue.
"""

import numpy as np
import ml_dtypes

import concourse.bass as bass
import concourse.tile as tile
import concourse.mybir as mybir
from concourse import bacc

F32 = mybir.dt.float32
BF16 = mybir.dt.bfloat16
BF16_NP = ml_dtypes.bfloat16


def impulse_response(c, s_out, tail_tol=1e-4):
    """Return (h, nb) with (nb-1)*128 taps covering the response."""
    AR = len(c)
    c = np.asarray(c, np.float64)
    h = np.zeros(s_out, np.float64)
    h[0] = 1.0
    for j in range(1, s_out):
        acc = 0.0
        for i in range(AR):
            if j - 1 - i >= 0:
                acc += c[i] * h[j - 1 - i]
        h[j] = acc
    L = 128
    while L < s_out and np.abs(h[L:]).sum() > tail_tol:
        L += 128
    # nb = number of 128-wide band blocks per output tile: the in-tile block
    # (j=0) plus one per preceding input tile the L-tap history reaches into.
    return h, L // 128 + 1


def band_blocks(h, nb):
    """Hb [128, nb*128]: block jj (for input-tile offset j = jj - (nb-1))
    has Hb[k_rel, t_rel] = h[t_rel - k_rel - 128*j] (0 <= lag < (nb-1)*128)."""
    L = (nb - 1) * 128
    a = np.arange(128)[:, None]   # k_rel
    b = np.arange(128)[None, :]   # t_rel
    blocks = []
    for jj in range(nb):
        j = jj - (nb - 1)
        lag = b - a - 128 * j
        m = (lag >= 0) & (lag < L)
        blk = np.where(m, np.take(np.pad(h[:L], (0, 1)), np.clip(lag, 0, L)), 0.0)
        blocks.append(blk)
    return np.concatenate(blocks, axis=1)


def host_prepare(W1, b1, W2, b2, W3, b3, ar_coef, S_IN, S_OUT, H):
    """Small device tensors in exactly the SBUF layout used, bf16."""
    n_s = S_IN // 128
    # W1l[p, k*H + h] = W1[h, k*128 + p]  (lhsT tiles for layer 1)
    W1l = np.ascontiguousarray(
        W1.reshape(H, n_s, 128).transpose(2, 1, 0).reshape(128, n_s * H)
    )
    h, nb = impulse_response(ar_coef, S_OUT)
    return {
        "W1l": W1l.astype(BF16_NP),
        "W2l": np.ascontiguousarray(W2.T).astype(BF16_NP),   # [H_in, H_out]
        "W3l": np.ascontiguousarray(W3.T).astype(BF16_NP),   # [H, S_OUT]
        "b1c": np.ascontiguousarray(b1.reshape(H, 1), dtype=np.float32),
        "b2c": np.ascontiguousarray(b2.reshape(H, 1), dtype=np.float32),
        "b3m": np.ascontiguousarray(b3.reshape(S_OUT // 128, 128).T,
                                    dtype=np.float32),       # [128, n_t]
        "Hb": band_blocks(h, nb).astype(BF16_NP),
    }, nb


def build_kernel(B_shard, S_IN, S_OUT, H, nb):
    P = 128
    NC = 2                        # batch chunks
    CW = B_shard // NC            # chunk width (free dim of every op)
    assert H == P and CW == 512
    n_s = S_IN // P               # 32 input k-tiles
    n_t = S_OUT // P              # 32 output t-tiles

    nc = bacc.Bacc(trn_type="TRN2", target_bir_lowering=False, debug=False)

    xT_d = nc.dram_tensor("xT", [NC, S_IN, CW], BF16, kind="ExternalInput").ap()
    nT_d = nc.dram_tensor("nT", [NC, S_OUT, CW], BF16, kind="ExternalInput").ap()
    W1_d = nc.dram_tensor("W1l", [P, n_s * H], BF16, kind="ExternalInput").ap()
    W2_d = nc.dram_tensor("W2l", [H, H], BF16, kind="ExternalInput").ap()
    W3_d = nc.dram_tensor("W3l", [H, S_OUT], BF16, kind="ExternalInput").ap()
    b1_d = nc.dram_tensor("b1c", [H, 1], F32, kind="ExternalInput").ap()
    b2_d = nc.dram_tensor("b2c", [H, 1], F32, kind="ExternalInput").ap()
    b3_d = nc.dram_tensor("b3m", [P, n_t], F32, kind="ExternalInput").ap()
    Hb_d = nc.dram_tensor("Hb", [P, nb * P], BF16, kind="ExternalInput").ap()
    out_d = nc.dram_tensor("outT", [NC, S_OUT, CW], BF16,
                           kind="ExternalOutput").ap()

    with tile.TileContext(nc) as tc:
        with tc.tile_pool(name="const", bufs=1) as cpool:
            # sync queue: W1 first (needed first), then x/noise tiles below.
            W1s = cpool.tile([P, n_s * H], BF16, tag="w1")
            nc.sync.dma_start(W1s[:], W1_d[:])
            # scalar queue: small consts, then it helps carry chunk-A x.
            W2s = cpool.tile([H, H], BF16, tag="w2")
            nc.scalar.dma_start(W2s[:], W2_d[:])
            b1s = cpool.tile([H, 1], F32, tag="b1")
            nc.scalar.dma_start(b1s[:], b1_d[:])
            b2s = cpool.tile([H, 1], F32, tag="b2")
            nc.scalar.dma_start(b2s[:], b2_d[:])
            # t-loop consts go on the gpsimd queue (idle until stores begin):
            # on scalar they would delay the chunk-A x tiles behind 1.2MB.
            b3s = cpool.tile([P, n_t], F32, tag="b3")
            nc.gpsimd.dma_start(b3s[:], b3_d[:])
            Hbs = cpool.tile([P, nb * P], BF16, tag="hb")
            nc.gpsimd.dma_start(Hbs[:], Hb_d[:])
            W3s = cpool.tile([H, S_OUT], BF16, tag="w3")
            nc.gpsimd.dma_start(W3s[:], W3_d[:])

            with (
                tc.tile_pool(name="warm", bufs=1) as wpool,
                tc.tile_pool(name="xT", bufs=n_s // 4) as xTp,
                tc.tile_pool(name="nT", bufs=4) as nTp,
                tc.tile_pool(name="act", bufs=2) as actp,
                tc.tile_pool(name="th", bufs=4) as thp,
                tc.tile_pool(name="outT", bufs=4) as outp,
                tc.tile_pool(name="psA", bufs=3, space="PSUM") as psA,
                tc.tile_pool(name="psB", bufs=4, space="PSUM") as psB,
            ):
                # ---- PE warm-up: the HAM clock gate defaults the PE array
                # to 1.2 GHz and only releases 2.4 GHz after ~3.4us of
                # sustained matmul activity; it re-throttles after ~3.4us
                # idle.  The first real matmul cannot start until W1+x
                # arrive (~13us: framework init + DMA), so keep the PE busy
                # on a zeroed scratch tile until then -- otherwise the whole
                # layer-1 phase runs at half clock.
                wsrc = wpool.tile([P, 4 * P], BF16, tag="wsrc")
                nc.vector.memset(wsrc[:], 0.0)
                wsnk = wpool.tile([P, 4], F32, tag="wsnk")
                psw = psB.tile([P, CW], F32, tag="ps", name="psw")
                for i in range(22):
                    nc.tensor.matmul(psw[:], wsrc[:, :P], wsrc[:])
                nc.vector.tensor_copy(wsnk[:], psw[:, :4])

                # ---- chunked input loads: 4 k-tiles per DMA (512KB)
                xts = {}

                def load_x4(c, g, eng):
                    t = xTp.tile([P, 4, CW], BF16, tag="xt", name=f"xt{c}_{g}")
                    src = xT_d[c, g * 4 * P:(g + 1) * 4 * P, :].rearrange(
                        "(blk p) f -> p blk f", p=P)
                    eng.dma_start(t[:], src)
                    xts[c, g] = t

                def xt(c, k):
                    return xts[c, k // 4][:, k % 4, :]

                ntm = {}

                def load_n4(c, g):
                    t = nTp.tile([P, 4, CW], BF16, tag="nt", name=f"nt{c}_{g}")
                    src = nT_d[c, g * 4 * P:(g + 1) * 4 * P, :].rearrange(
                        "(blk p) f -> p blk f", p=P)
                    nc.sync.dma_start(t[:], src)
                    ntm[c, g] = t

                def nt(c, m):
                    return ntm[c, m // 4][:, m % 4, :]

                # chunk A x: alternate sync/scalar so one queue's trigger
                # rate doesn't cap the wire.
                for g in range(n_s // 4):
                    load_x4(0, g, nc.sync if g % 2 == 0 else nc.scalar)
                # chunk A noise + chunk B x, interleaved on sync: both are
                # consumed at ~1 tile per t-loop-A step.
                load_n4(0, 0)
                for g in range(n_s // 4):
                    load_x4(1, g, nc.sync)
                    if g + 1 < 8:
                        load_n4(0, g + 1)

                def layer12(c, psh1):
                    h1T = actp.tile([H, CW], BF16, tag="act", name=f"h1T{c}")
                    nc.scalar.activation(
                        h1T[:], psh1[:], mybir.ActivationFunctionType.Relu,
                        bias=b1s[:],
                    )
                    psh2 = psA.tile([H, CW], F32, tag="psA", name=f"psh2{c}")
                    nc.tensor.matmul(psh2[:], W2s[:], h1T[:])
                    h2T = actp.tile([H, CW], BF16, tag="act", name=f"h2T{c}")
                    nc.scalar.activation(
                        h2T[:], psh2[:], mybir.ActivationFunctionType.Relu,
                        bias=b2s[:],
                    )
                    return h2T

                # ---- L1(A) as one dense run; L1(B) is interleaved into
                # t-loop-A below (1 matmul per t-tile) to use the PE slack
                # while that loop is wire-paced.
                psh1a = psA.tile([H, CW], F32, tag="psA", name="psh1a")
                for k in range(n_s):
                    nc.tensor.matmul(
                        psh1a[:], W1s[:, k * H:(k + 1) * H], xt(0, k),
                        start=(k == 0), stop=(k == n_s - 1),
                    )
                h2Ta = layer12(0, psh1a)
                psh1b = psA.tile([H, CW], F32, tag="psA", name="psh1b")

                def t_loop(c, h2T, extra=None):
                    for m in range(n_t):
                        if m % 4 == 0:
                            g = (m + 8) // 4
                            if c == 0:
                                # chunk-A groups were all queued up front
                                # (pool backpressure self-paces them)
                                if g >= n_t // 4:
                                    load_n4(1, g - n_t // 4)
                            elif g < n_t // 4:
                                load_n4(c, g)
                        jlist = [j for j in range(-(nb - 1), 1) if m + j >= 0]
                        psc = psB.tile([P, CW], F32, tag="ps",
                                       name=f"psc{c}_{m}")
                        for i, j in enumerate(jlist):
                            jj = j + nb - 1
                            nc.tensor.matmul(
                                psc[:], Hbs[:, jj * P:(jj + 1) * P],
                                nt(c, m + j),
                                start=(i == 0), stop=(i == len(jlist) - 1),
                            )
                        psm = psA.tile([P, CW], F32, tag="psA",
                                       name=f"psm{c}_{m}")
                        nc.tensor.matmul(
                            psm[:], W3s[:, m * P:(m + 1) * P], h2T[:],
                        )
                        if extra is not None:
                            extra(m)
                        th = thp.tile([P, CW], F32, tag="th")
                        nc.scalar.activation(
                            th[:], psm[:], mybir.ActivationFunctionType.Tanh,
                            bias=b3s[:, m:m + 1],
                        )
                        if m % 2 == 0:
                            ot = outp.tile([P, 2, CW], BF16, tag="ot",
                                           name=f"ot{c}_{m // 2}")
                        nc.vector.tensor_add(ot[:, m % 2, :], th[:], psc[:])
                        if m % 2 == 1:
                            dst = out_d[c, (m - 1) * P:(m + 1) * P, :].rearrange(
                                "(blk p) f -> p blk f", p=P)
                            nc.gpsimd.dma_start(dst, ot[:])

                def l1b_step(m):
                    nc.tensor.matmul(
                        psh1b[:], W1s[:, m * H:(m + 1) * H], xt(1, m),
                        start=(m == 0), stop=(m == n_s - 1),
                    )

                t_loop(0, h2Ta, extra=l1b_step)
                h2Tb = layer12(1, psh1b)
                t_loop(1, h2Tb)

    nc.compile()
    return nc


# ---------------------------------------------------------------------------
# Self-contained kernel() entry point (the graded contract).
# ---------------------------------------------------------------------------

N_CORES = 8
_B, _S_IN, _S_OUT, _H, _AR = 8192, 4096, 4096, 128, 7
_CW = 512

_CACHE = {}


def _prep_and_build(inputs):
    dev, nb = host_prepare(
        np.asarray(inputs["W1"], np.float32), np.asarray(inputs["b1"], np.float32),
        np.asarray(inputs["W2"], np.float32), np.asarray(inputs["b2"], np.float32),
        np.asarray(inputs["W3"], np.float32), np.asarray(inputs["b3"], np.float32),
        np.asarray(inputs["ar_coef"], np.float32),
        _S_IN, _S_OUT, _H,
    )
    B_total = inputs["x"].shape[0]
    B_shard = B_total // N_CORES
    key = (B_shard, nb)
    if key not in _CACHE:
        _CACHE[key] = build_kernel(B_shard, _S_IN, _S_OUT, _H, nb)
    return _CACHE[key], dev, B_shard


def _chunked_T(a, B_shard):
    """[B_shard, S] fp32 -> [2, S, B_shard//2] bf16 (transposed chunks)."""
    aT = a.astype(BF16_NP).T                      # [S, B_shard]
    cw = B_shard // 2
    return np.ascontiguousarray(
        np.stack([aT[:, :cw], aT[:, cw:]]))


def _in_maps(inputs, dev, B_shard):
    x = np.asarray(inputs["x"], np.float32)
    noise_m = np.asarray(inputs["noise"], np.float32).copy()
    noise_m[:, :_AR] = 0.0
    maps = []
    for c in range(N_CORES):
        sl = slice(c * B_shard, (c + 1) * B_shard)
        m = {"xT": _chunked_T(x[sl], B_shard),
             "nT": _chunked_T(noise_m[sl], B_shard)}
        m.update(dev)
        maps.append(m)
    return maps


def kernel(**inputs):
    nc, dev, B_shard = _prep_and_build(inputs)
    maps = _in_maps(inputs, dev, B_shard)
    import concourse.bass_utils as bass_utils

    res = bass_utils.run_bass_kernel_spmd(
        nc, maps, core_ids=list(range(N_CORES)), trace=False
    )
    shards = []
    for c in range(N_CORES):
        o = np.asarray(res.results[c]["outT"])    # [2, S_OUT, CW] bf16
        shards.append(np.concatenate([o[0].T, o[1].T], axis=0))
    return np.concatenate(shards, axis=0).astype(np.float32)


def run_traced(inputs):
    """Profiled run (NTFF -> neuron-profile) for the local test harness."""
    import contextlib
    import ctypes
    import sys as _sys
    import types as _types

    so = "/opt/axon/libaxon_pjrt.so"
    if "antenv.axon_hooks" not in _sys.modules:
        try:
            lib2 = ctypes.CDLL(so)
            lib2.axon_start_nrt_profile.argtypes = [
                ctypes.POINTER(ctypes.c_int64), ctypes.c_size_t]
            lib2.axon_start_nrt_profile.restype = ctypes.c_int64
            lib2.axon_stop_nrt_profile.argtypes = [ctypes.c_char_p]
            lib2.axon_stop_nrt_profile.restype = ctypes.c_int64

            @contextlib.contextmanager
            def _hook(output_dir, device_ids):
                import jax
                jax.devices()
                if device_ids:
                    ids_arr = (ctypes.c_int64 * len(device_ids))(*device_ids)
                    rc = lib2.axon_start_nrt_profile(ids_arr, len(device_ids))
                else:
                    rc = lib2.axon_start_nrt_profile(None, 0)
                if rc != 0:
                    raise RuntimeError(f"axon_start_nrt_profile rc={rc}")
                try:
                    yield
                finally:
                    lib2.axon_stop_nrt_profile(str(output_dir).encode())

            mod = _types.ModuleType("antenv.axon_hooks")
            mod.get_axon_ntff_profile_hook = lambda: _hook
            mod.set_axon_ntff_profile_hook = lambda h: None
            _sys.modules["antenv.axon_hooks"] = mod
        except OSError:
            pass
    import concourse.bass_utils as bass_utils
    bass_utils.upload_artifacts = lambda tmpdir: tmpdir

    nc, dev, B_shard = _prep_and_build(inputs)
    maps = _in_maps(inputs, dev, B_shard)
    return bass_utils.run_bass_kernel_spmd(
        nc, maps, core_ids=list(range(N_CORES)), trace=True, trace_cores=[0]
    )


# revision 14
# speedup vs baseline: 2.2390x; 1.0179x over previous
"""ARGenerator TRN2 kernel.

Math (per batch row b):
  h1 = relu(x @ W1.T + b1); h2 = relu(h1 @ W2.T + b2)
  mlp = tanh(h2 @ W3.T + b3)
  ar[t] = noise[t] + sum_i c[i] * ar[t-1-i]  (zero-init, t >= 7; 0 for t < 7)
  out = mlp + ar

The AR recurrence is linear time-invariant -> ar = conv(noise_masked, h)
with h the (geometrically decaying) impulse response, truncated at
(nb-1)*128 taps.  The conv becomes nb banded 128x128 Toeplitz matmuls
per output time-tile, fully parallel over time.

Layout strategy (pure data parallel over 8 cores, B_shard = 1024):
  Everything runs in bf16 (tolerance is 2e-2; bf16 keeps us ~5e-3) and
  in TRANSPOSED activation layout [feature/time on partitions, batch on
  free dim].  The host pre-transposes x and noise shards, so the device
  does plain contiguous DMA loads and zero PE transposes.  The output
  is produced transposed (which lets the b3 bias + tanh fuse into one
  ACT instruction with a per-partition bias) and the host transposes it
  back.

Pipeline: the batch shard is processed in 2 chunks of 512 so that
chunk B's input DMA and layer-1 matmuls overlap chunk A's t-loop
(the t-loop is wire-paced, leaving PE idle slack that exactly fits
L1(B)); chunk A's stores likewise overlap chunk B's loads.  Per-core
wire traffic: 1MB W1 + 1.1MB consts + 8MB xT + 8MB nT + 8MB out.

Per-chunk phases:
  1. L1: psh1[h, b] += W1l_k.T @ xT_k over 32 k-tiles; ACT relu+b1.
  2. L2: one matmul + ACT relu+b2 -> h2T [128, 512] bf16.
  3. For each t-tile m (32):
       psc = sum_j Hb_j.T @ nT[m+j]     (banded conv, nb matmuls)
       psm = W3l[:, m].T @ h2T          (one matmul)
       th  = tanh(psm + b3[m])          (ACT, per-partition bias)
       out = th + psc  (DVE, the only engine that can read PSUM)
       -> bf16, stores merged x2 on the gpsimd queue.
"""

import numpy as np
import ml_dtypes

import concourse.bass as bass
import concourse.tile as tile
import concourse.mybir as mybir
from concourse import bacc

F32 = mybir.dt.float32
BF16 = mybir.dt.bfloat16
BF16_NP = ml_dtypes.bfloat16


def impulse_response(c, s_out, tail_tol=1e-4):
    """Return (h, nb) with (nb-1)*128 taps covering the response."""
    AR = len(c)
    c = np.asarray(c, np.float64)
    h = np.zeros(s_out, np.float64)
    h[0] = 1.0
    for j in range(1, s_out):
        acc = 0.0
        for i in range(AR):
            if j - 1 - i >= 0:
                acc += c[i] * h[j - 1 - i]
        h[j] = acc
    L = 128
    while L < s_out and np.abs(h[L:]).sum() > tail_tol:
        L += 128
    # nb = number of 128-wide band blocks per output tile: the in-tile block
    # (j=0) plus one per preceding input tile the L-tap history reaches into.
    return h, L // 128 + 1


def band_blocks(h, nb):
    """Hb [128, nb*128]: block jj (for input-tile offset j = jj - (nb-1))
    has Hb[k_rel, t_rel] = h[t_rel - k_rel - 128*j] (0 <= lag < (nb-1)*128)."""
    L = (nb - 1) * 128
    a = np.arange(128)[:, None]   # k_rel
    b = np.arange(128)[None, :]   # t_rel
    blocks = []
    for jj in range(nb):
        j = jj - (nb - 1)
        lag = b - a - 128 * j
        m = (lag >= 0) & (lag < L)
        blk = np.where(m, np.take(np.pad(h[:L], (0, 1)), np.clip(lag, 0, L)), 0.0)
        blocks.append(blk)
    return np.concatenate(blocks, axis=1)


def host_prepare(W1, b1, W2, b2, W3, b3, ar_coef, S_IN, S_OUT, H):
    """Small device tensors in exactly the SBUF layout used, bf16."""
    n_s = S_IN // 128
    # W1l[p, k*H + h] = W1[h, k*128 + p]  (lhsT tiles for layer 1)
    W1l = np.ascontiguousarray(
        W1.reshape(H, n_s, 128).transpose(2, 1, 0).reshape(128, n_s * H)
    )
    h, nb = impulse_response(ar_coef, S_OUT)
    return {
        "W1l": W1l.astype(BF16_NP),
        "W2l": np.ascontiguousarray(W2.T).astype(BF16_NP),   # [H_in, H_out]
        "W3l": np.ascontiguousarray(W3.T).astype(BF16_NP),   # [H, S_OUT]
        "b1c": np.ascontiguousarray(b1.reshape(H, 1), dtype=np.float32),
        "b2c": np.ascontiguousarray(b2.reshape(H, 1), dtype=np.float32),
        "b3m": np.ascontiguousarray(b3.reshape(S_OUT // 128, 128).T,
                                    dtype=np.float32),       # [128, n_t]
        "Hb": band_blocks(h, nb).astype(BF16_NP),
    }, nb


def build_kernel(B_shard, S_IN, S_OUT, H, nb):
    P = 128
    NC = 2                        # batch chunks
    CW = B_shard // NC            # chunk width (free dim of every op)
    assert H == P and CW == 512
    n_s = S_IN // P               # 32 input k-tiles
    n_t = S_OUT // P              # 32 output t-tiles

    nc = bacc.Bacc(trn_type="TRN2", target_bir_lowering=False, debug=False)

    xT_d = nc.dram_tensor("xT", [NC, S_IN, CW], BF16, kind="ExternalInput").ap()
    nT_d = nc.dram_tensor("nT", [NC, S_OUT, CW], BF16, kind="ExternalInput").ap()
    W1_d = nc.dram_tensor("W1l", [P, n_s * H], BF16, kind="ExternalInput").ap()
    W2_d = nc.dram_tensor("W2l", [H, H], BF16, kind="ExternalInput").ap()
    W3_d = nc.dram_tensor("W3l", [H, S_OUT], BF16, kind="ExternalInput").ap()
    b1_d = nc.dram_tensor("b1c", [H, 1], F32, kind="ExternalInput").ap()
    b2_d = nc.dram_tensor("b2c", [H, 1], F32, kind="ExternalInput").ap()
    b3_d = nc.dram_tensor("b3m", [P, n_t], F32, kind="ExternalInput").ap()
    Hb_d = nc.dram_tensor("Hb", [P, nb * P], BF16, kind="ExternalInput").ap()
    out_d = nc.dram_tensor("outT", [NC, S_OUT, CW], BF16,
                           kind="ExternalOutput").ap()

    with tile.TileContext(nc) as tc:
        with tc.tile_pool(name="const", bufs=1) as cpool:
            # W1 split into 4 tiles across both queues: dependency tracking
            # is per-tile, so a monolithic W1 would stall layer-1 k=0 until
            # the whole 1MB lands.
            W1t = []
            for i in range(4):
                w = cpool.tile([P, (n_s // 4) * H], BF16, tag=f"w1_{i}")
                (nc.sync if i % 2 == 0 else nc.scalar).dma_start(
                    w[:], W1_d[:, i * (n_s // 4) * H:(i + 1) * (n_s // 4) * H])
                W1t.append(w)

            def W1sl(k):
                return W1t[k // 8][:, (k % 8) * H:(k % 8 + 1) * H]
            # scalar queue: small consts, then it helps carry chunk-A x.
            W2s = cpool.tile([H, H], BF16, tag="w2")
            nc.scalar.dma_start(W2s[:], W2_d[:])
            b1s = cpool.tile([H, 1], F32, tag="b1")
            nc.scalar.dma_start(b1s[:], b1_d[:])
            b2s = cpool.tile([H, 1], F32, tag="b2")
            nc.scalar.dma_start(b2s[:], b2_d[:])
            # t-loop consts go on the gpsimd queue (idle until stores begin):
            # on scalar they would delay the chunk-A x tiles behind 1.2MB.
            b3s = cpool.tile([P, n_t], F32, tag="b3")
            nc.gpsimd.dma_start(b3s[:], b3_d[:])
            Hbs = cpool.tile([P, nb * P], BF16, tag="hb")
            nc.gpsimd.dma_start(Hbs[:], Hb_d[:])
            W3s = cpool.tile([H, S_OUT], BF16, tag="w3")
            nc.gpsimd.dma_start(W3s[:], W3_d[:])

            with (
                tc.tile_pool(name="warm", bufs=1) as wpool,
                tc.tile_pool(name="xT", bufs=n_s // 4) as xTp,
                tc.tile_pool(name="nT", bufs=4) as nTp,
                tc.tile_pool(name="act", bufs=2) as actp,
                tc.tile_pool(name="th", bufs=4) as thp,
                tc.tile_pool(name="outT", bufs=4) as outp,
                tc.tile_pool(name="psA", bufs=3, space="PSUM") as psA,
                tc.tile_pool(name="psB", bufs=4, space="PSUM") as psB,
            ):
                # ---- PE warm-up: the HAM clock gate defaults the PE array
                # to 1.2 GHz and only releases 2.4 GHz after ~3.4us of
                # sustained matmul activity; it re-throttles after ~3.4us
                # idle.  The first real matmul cannot start until W1+x
                # arrive (~13us: framework init + DMA), so keep the PE busy
                # on a zeroed scratch tile until then -- otherwise the whole
                # layer-1 phase runs at half clock.
                wsrc = wpool.tile([P, 4 * P], BF16, tag="wsrc")
                nc.vector.memset(wsrc[:], 0.0)
                wsnk = wpool.tile([P, 4], F32, tag="wsnk")
                psw = psB.tile([P, CW], F32, tag="ps", name="psw")
                for i in range(16):
                    nc.tensor.matmul(psw[:], wsrc[:, :P], wsrc[:])
                nc.vector.tensor_copy(wsnk[:], psw[:, :4])

                # ---- chunked input loads: 4 k-tiles per DMA (512KB)
                xts = {}

                def load_x4(c, g, eng):
                    t = xTp.tile([P, 4, CW], BF16, tag="xt", name=f"xt{c}_{g}")
                    src = xT_d[c, g * 4 * P:(g + 1) * 4 * P, :].rearrange(
                        "(blk p) f -> p blk f", p=P)
                    eng.dma_start(t[:], src)
                    xts[c, g] = t

                def xt(c, k):
                    return xts[c, k // 4][:, k % 4, :]

                ntm = {}

                def load_n4(c, g):
                    t = nTp.tile([P, 4, CW], BF16, tag="nt", name=f"nt{c}_{g}")
                    src = nT_d[c, g * 4 * P:(g + 1) * 4 * P, :].rearrange(
                        "(blk p) f -> p blk f", p=P)
                    nc.sync.dma_start(t[:], src)
                    ntm[c, g] = t

                def nt(c, m):
                    return ntm[c, m // 4][:, m % 4, :]

                # chunk A x: alternate sync/scalar so one queue's trigger
                # rate doesn't cap the wire.
                for g in range(n_s // 4):
                    load_x4(0, g, nc.sync if g % 2 == 0 else nc.scalar)
                # chunk A noise + chunk B x, interleaved on sync: both are
                # consumed at ~1 tile per t-loop-A step.
                load_n4(0, 0)
                for g in range(n_s // 4):
                    load_x4(1, g, nc.sync)
                    if g + 1 < 8:
                        load_n4(0, g + 1)

                def layer12(c, psh1):
                    h1T = actp.tile([H, CW], BF16, tag="act", name=f"h1T{c}")
                    nc.scalar.activation(
                        h1T[:], psh1[:], mybir.ActivationFunctionType.Relu,
                        bias=b1s[:],
                    )
                    psh2 = psA.tile([H, CW], F32, tag="psA", name=f"psh2{c}")
                    nc.tensor.matmul(psh2[:], W2s[:], h1T[:])
                    h2T = actp.tile([H, CW], BF16, tag="act", name=f"h2T{c}")
                    nc.scalar.activation(
                        h2T[:], psh2[:], mybir.ActivationFunctionType.Relu,
                        bias=b2s[:],
                    )
                    return h2T

                # ---- L1(A) as one dense run; L1(B) is interleaved into
                # t-loop-A below (1 matmul per t-tile) to use the PE slack
                # while that loop is wire-paced.
                psh1a = psA.tile([H, CW], F32, tag="psA", name="psh1a")
                for k in range(n_s):
                    nc.tensor.matmul(
                        psh1a[:], W1sl(k), xt(0, k),
                        start=(k == 0), stop=(k == n_s - 1),
                    )
                h2Ta = layer12(0, psh1a)
                psh1b = psA.tile([H, CW], F32, tag="psA", name="psh1b")

                def t_loop(c, h2T, extra=None):
                    for m in range(n_t):
                        if m % 4 == 0:
                            g = (m + 8) // 4
                            if c == 0:
                                # chunk-A groups were all queued up front
                                # (pool backpressure self-paces them)
                                if g >= n_t // 4:
                                    load_n4(1, g - n_t // 4)
                            elif g < n_t // 4:
                                load_n4(c, g)
                        jlist = [j for j in range(-(nb - 1), 1) if m + j >= 0]
                        psc = psB.tile([P, CW], F32, tag="ps",
                                       name=f"psc{c}_{m}")
                        for i, j in enumerate(jlist):
                            jj = j + nb - 1
                            nc.tensor.matmul(
                                psc[:], Hbs[:, jj * P:(jj + 1) * P],
                                nt(c, m + j),
                                start=(i == 0), stop=(i == len(jlist) - 1),
                            )
                        psm = psA.tile([P, CW], F32, tag="psA",
                                       name=f"psm{c}_{m}")
                        nc.tensor.matmul(
                            psm[:], W3s[:, m * P:(m + 1) * P], h2T[:],
                        )
                        if extra is not None:
                            extra(m)
                        th = thp.tile([P, CW], F32, tag="th")
                        nc.scalar.activation(
                            th[:], psm[:], mybir.ActivationFunctionType.Tanh,
                            bias=b3s[:, m:m + 1],
                        )
                        if m % 2 == 0:
                            ot = outp.tile([P, 2, CW], BF16, tag="ot",
                                           name=f"ot{c}_{m // 2}")
                        nc.vector.tensor_add(ot[:, m % 2, :], th[:], psc[:])
                        if m % 2 == 1:
                            dst = out_d[c, (m - 1) * P:(m + 1) * P, :].rearrange(
                                "(blk p) f -> p blk f", p=P)
                            nc.gpsimd.dma_start(dst, ot[:])

                def l1b_step(m):
                    nc.tensor.matmul(
                        psh1b[:], W1sl(m), xt(1, m),
                        start=(m == 0), stop=(m == n_s - 1),
                    )

                t_loop(0, h2Ta, extra=l1b_step)
                h2Tb = layer12(1, psh1b)
                t_loop(1, h2Tb)

    nc.compile()
    return nc


# ---------------------------------------------------------------------------
# Self-contained kernel() entry point (the graded contract).
# ---------------------------------------------------------------------------

N_CORES = 8
_B, _S_IN, _S_OUT, _H, _AR = 8192, 4096, 4096, 128, 7
_CW = 512

_CACHE = {}


def _prep_and_build(inputs):
    dev, nb = host_prepare(
        np.asarray(inputs["W1"], np.float32), np.asarray(inputs["b1"], np.float32),
        np.asarray(inputs["W2"], np.float32), np.asarray(inputs["b2"], np.float32),
        np.asarray(inputs["W3"], np.float32), np.asarray(inputs["b3"], np.float32),
        np.asarray(inputs["ar_coef"], np.float32),
        _S_IN, _S_OUT, _H,
    )
    B_total = inputs["x"].shape[0]
    B_shard = B_total // N_CORES
    key = (B_shard, nb)
    if key not in _CACHE:
        _CACHE[key] = build_kernel(B_shard, _S_IN, _S_OUT, _H, nb)
    return _CACHE[key], dev, B_shard


def _chunked_T(a, B_shard):
    """[B_shard, S] fp32 -> [2, S, B_shard//2] bf16 (transposed chunks)."""
    aT = a.astype(BF16_NP).T                      # [S, B_shard]
    cw = B_shard // 2
    return np.ascontiguousarray(
        np.stack([aT[:, :cw], aT[:, cw:]]))


def _in_maps(inputs, dev, B_shard):
    x = np.asarray(inputs["x"], np.float32)
    noise_m = np.asarray(inputs["noise"], np.float32).copy()
    noise_m[:, :_AR] = 0.0
    maps = []
    for c in range(N_CORES):
        sl = slice(c * B_shard, (c + 1) * B_shard)
        m = {"xT": _chunked_T(x[sl], B_shard),
             "nT": _chunked_T(noise_m[sl], B_shard)}
        m.update(dev)
        maps.append(m)
    return maps


def kernel(**inputs):
    nc, dev, B_shard = _prep_and_build(inputs)
    maps = _in_maps(inputs, dev, B_shard)
    import concourse.bass_utils as bass_utils

    res = bass_utils.run_bass_kernel_spmd(
        nc, maps, core_ids=list(range(N_CORES)), trace=False
    )
    shards = []
    for c in range(N_CORES):
        o = np.asarray(res.results[c]["outT"])    # [2, S_OUT, CW] bf16
        shards.append(np.concatenate([o[0].T, o[1].T], axis=0))
    return np.concatenate(shards, axis=0).astype(np.float32)


def run_traced(inputs):
    """Profiled run (NTFF -> neuron-profile) for the local test harness."""
    import contextlib
    import ctypes
    import sys as _sys
    import types as _types

    so = "/opt/axon/libaxon_pjrt.so"
    if "antenv.axon_hooks" not in _sys.modules:
        try:
            lib2 = ctypes.CDLL(so)
            lib2.axon_start_nrt_profile.argtypes = [
                ctypes.POINTER(ctypes.c_int64), ctypes.c_size_t]
            lib2.axon_start_nrt_profile.restype = ctypes.c_int64
            lib2.axon_stop_nrt_profile.argtypes = [ctypes.c_char_p]
            lib2.axon_stop_nrt_profile.restype = ctypes.c_int64

            @contextlib.contextmanager
            def _hook(output_dir, device_ids):
                import jax
                jax.devices()
                if device_ids:
                    ids_arr = (ctypes.c_int64 * len(device_ids))(*device_ids)
                    rc = lib2.axon_start_nrt_profile(ids_arr, len(device_ids))
                else:
                    rc = lib2.axon_start_nrt_profile(None, 0)
                if rc != 0:
                    raise RuntimeError(f"axon_start_nrt_profile rc={rc}")
                try:
                    yield
                finally:
                    lib2.axon_stop_nrt_profile(str(output_dir).encode())

            mod = _types.ModuleType("antenv.axon_hooks")
            mod.get_axon_ntff_profile_hook = lambda: _hook
            mod.set_axon_ntff_profile_hook = lambda h: None
            _sys.modules["antenv.axon_hooks"] = mod
        except OSError:
            pass
    import concourse.bass_utils as bass_utils
    bass_utils.upload_artifacts = lambda tmpdir: tmpdir

    nc, dev, B_shard = _prep_and_build(inputs)
    maps = _in_maps(inputs, dev, B_shard)
    return bass_utils.run_bass_kernel_spmd(
        nc, maps, core_ids=list(range(N_CORES)), trace=True, trace_cores=[0]
    )
